# revision 2
# baseline (speedup 1.0000x reference)
"""MLA (multi-head latent attention) prefill kernel for 8 Trainium2 NeuronCores.

Sharding: data-parallel over (batch, query-chunk) for the q path, attention
and o_proj; the KV path (kv-mix, k_nope = kv_lat@k_up^T, v_lat = kv_lat@v_up^T)
is sharded by key-chunk across the 4 cores of each batch and exchanged via
three AllGathers (rope plane 0.5MB, k_nope 8MB, v_lat 8MB) on replica groups
[[0,1,2,3],[4,5,6,7]]. Collectives run on TOPSP/SDMA silicon so they overlap
with PE compute; all q-side work (q-mix, q_nope, q_rope for all heads) is
issued before the first attention score matmul so the gathers have ~100us to
land. Per-core matmul work drops from ~37.6 GF to ~27.2 GF.

Structure:
 - Segment B': kv-mix over own 512 keys only (the same hidden rows as the
   q chunk, so hst4 cross-chunk hidden-state DMAs are gone), LN + RoPE,
   transposes, then local k_nope (all 16 heads x own keys) and local v_lat
   (all 4 groups x own keys); evict + AllGather each.
 - Segment A: q-mix matmuls + LayerNorm (unchanged from baseline).
 - Segment C: q latent transposes (batched, 6/bank).
 - Phase 1.5: q_rope for all 4 head groups + q_nope for all 16 heads,
   buffered in SBUF (3MB) so the head loop has no q-side dependencies.
 - Phase 2 head loop: scores contract 128+64 channels against DMA-loaded
   gathered k_nope^T/rope^T; softmax denominator via pair->quad->oct folds
   + ones-matmuls; attn@v_lat per head. All matmuls bf16 (fp32 PSUM).
 - Phase 3: o_proj in 4 quarter passes with double-buffered PSUM.
"""

import math
from contextlib import ExitStack

import numpy as np
from ml_dtypes import bfloat16

import concourse.bass as bass
import concourse.tile as tile
from concourse import bacc, mybir
from concourse.bass_utils import run_bass_kernel_spmd
from concourse.masks import make_identity

F32 = mybir.dt.float32
F32R = mybir.dt.float32r
BF16 = mybir.dt.bfloat16
AF = mybir.ActivationFunctionType
OP = mybir.AluOpType

B, S, D = 2, 2048, 2048
H = 16
LAT = 1536
R = 512
DN, DR, DV = 128, 64, 128
EPS = 1e-5
SCALE = 1.0 / math.sqrt(DN + DR)

P = 128
CH = 512
NQT = CH // P      # 4 q tiles per chunk
NKT = S // P       # 16 key tiles total
NKTC = CH // P     # 4 key tiles in own chunk
NDT = D // P
NLT = LAT // P

N_CORES = 8
G = 4              # ranks per replica group (one batch)
RG = [[0, 1, 2, 3], [4, 5, 6, 7]]


def _bcast_rows(t, n, length):
    return bass.AP(tensor=t, offset=0, ap=[[0, n], [1, length]])


def build_nc():
    nc = bacc.Bacc(None, target_bir_lowering=False, num_devices=N_CORES)

    hsqt = nc.dram_tensor("hsqt", [2, NDT, 2, P, P], BF16, kind="ExternalInput")
    wqa_t = nc.dram_tensor("wqa_t", [D, LAT], BF16, kind="ExternalInput")
    wqb_t = nc.dram_tensor("wqb_t", [LAT, H * DN], BF16, kind="ExternalInput")
    wqr_t = nc.dram_tensor("wqr_t", [LAT, H * DR], BF16, kind="ExternalInput")
    wkva_t = nc.dram_tensor("wkva_t", [D, R + DR], BF16, kind="ExternalInput")
    kup_t = nc.dram_tensor("kup_t", [R, H * DN], BF16, kind="ExternalInput")
    vup_t = nc.dram_tensor("vup_t", [R, H * DV], BF16, kind="ExternalInput")
    wo_t = nc.dram_tensor("wo_t", [H * DV, D], BF16, kind="ExternalInput")
    bqn_v = nc.dram_tensor("bqn_v", [H * DN], F32, kind="ExternalInput")
    bqr_v = nc.dram_tensor("bqr_v", [H * DR], F32, kind="ExternalInput")
    bkn_v = nc.dram_tensor("bkn_v", [H * DN], F32, kind="ExternalInput")
    bvv_v = nc.dram_tensor("bvv_v", [H * DV], F32, kind="ExternalInput")
    ones_in = nc.dram_tensor("ones_in", [P, P], F32R, kind="ExternalInput")
    ck_tab = nc.dram_tensor("ck_tab", [CH, DR // 2], F32, kind="ExternalInput")
    sk_tab = nc.dram_tensor("sk_tab", [CH, DR // 2], F32, kind="ExternalInput")
    cq_tab = nc.dram_tensor("cq_tab", [P, CH], F32, kind="ExternalInput")
    sq_tab = nc.dram_tensor("sq_tab", [P, CH], F32, kind="ExternalInput")
    out_c = nc.dram_tensor("out_c", [CH, D], F32, kind="ExternalOutput")

    # collective bounce buffers (internal DRAM)
    rp_in = nc.dram_tensor("rp_in", [P, CH], BF16)
    rp_out = nc.dram_tensor("rp_out", [G * P, CH], BF16)
    kn_in = nc.dram_tensor("kn_in", [H * P, CH], BF16)
    kn_out = nc.dram_tensor("kn_out", [G * H * P, CH], BF16)
    vl_in = nc.dram_tensor("vl_in", [CH, H * DV], BF16)
    vl_out = nc.dram_tensor("vl_out", [G * CH, H * DV], BF16)

    with tile.TileContext(nc) as tc, ExitStack() as octx:
        res = octx.enter_context(tc.tile_pool(name="res", bufs=1))
        ropeT = res.tile([P, S], BF16)       # gathered k_rope^T (rows dup'd)

        consts = octx.enter_context(tc.tile_pool(name="consts", bufs=1))
        ident = consts.tile([P, P], BF16)
        make_identity(nc, ident)
        ones_t = consts.tile([P, P], F32R)
        eps_t = consts.tile([P, 1], F32)
        nc.vector.memset(eps_t, EPS)
        cq_t = consts.tile([P, CH], F32)
        sq_t = consts.tile([P, CH], F32)
        bqn_t = consts.tile([P, H], F32)
        bqr_t = consts.tile([P, 8], F32)
        bkn_t = consts.tile([P, H], F32)
        bvv_bc = consts.tile([P, H * DV], F32)

        qstage = octx.enter_context(tc.tile_pool(name="qstage", bufs=1))
        qro_all = qstage.tile([P, G, 2, CH], BF16)
        qnope_all = qstage.tile([P, H, CH], BF16)

        wop = octx.enter_context(tc.tile_pool(name="wop", bufs=6))

        with ExitStack() as p1all:
            hsqp = p1all.enter_context(tc.tile_pool(name="hsqp", bufs=1))
            wkvp = p1all.enter_context(tc.tile_pool(name="wkvp", bufs=1))
            kvwp = p1all.enter_context(tc.tile_pool(name="kvwp", bufs=1))
            lnsp = p1all.enter_context(tc.tile_pool(name="lnsp", bufs=1))
            qproj = p1all.enter_context(tc.tile_pool(name="qproj", bufs=1))
            qlat_t = qproj.tile([P, NLT, CH], BF16)

            hsq_all = hsqp.tile([P, 2, NDT, 2, P], BF16)
            wkv_all = wkvp.tile([P, NDT, R + DR], BF16)
            ck_t = wkvp.tile([P, NKTC, DR // 2], F32)
            sk_t = wkvp.tile([P, NKTC, DR // 2], F32)
            kupT = kvwp.tile([P, 4, H * DN], BF16)
            vupT = kvwp.tile([P, 4, H * DV], BF16)
            qln_all = lnsp.tile([P, NQT, LAT], BF16)
            lnf_loc = lnsp.tile([P, NKTC, R], BF16)
            kro_loc = lnsp.tile([P, NKTC, P], BF16)

            # =============== segment B': kv-mix on own 512 keys ===============
            with ExitStack() as pB:
                mixp = pB.enter_context(tc.tile_pool(name="mixp", bufs=3))
                lnp = pB.enter_context(tc.tile_pool(name="lnp", bufs=2))
                psB = pB.enter_context(tc.tile_pool(name="psB", bufs=1, space="PSUM"))

                # interleave kv weights with own hidden-state tiles so the
                # first matmul can start after ~1.1MB of DMA
                for a in range(4):
                    nc.sync.dma_start(
                        wkv_all[:, 4 * a:4 * (a + 1), :],
                        wkva_t[512 * a:512 * (a + 1), :].rearrange(
                            "(t p) c -> p t c", p=P
                        ),
                    )
                    for pair in range(2):
                        nc.sync.dma_start(
                            hsq_all[:, pair, 4 * a:4 * (a + 1), :, :],
                            hsqt[pair, 4 * a:4 * (a + 1)].rearrange(
                                "d k p c -> p d k c"
                            ),
                        )
                nc.sync.dma_start(
                    ck_t[:], ck_tab.ap().rearrange("(t p) j -> p t j", p=P)
                )
                nc.sync.dma_start(
                    sk_t[:], sk_tab.ap().rearrange("(t p) j -> p t j", p=P)
                )

                # phase-2 weights streamed during kv-mix
                def _stream_weights(dt):
                    if dt % 4 == 1:
                        rc = dt // 4
                        nc.sync.dma_start(
                            kupT[:, rc, :], kup_t[rc * P:(rc + 1) * P, :]
                        )
                    elif dt % 4 == 3:
                        rc = dt // 4
                        nc.sync.dma_start(
                            vupT[:, rc, :], vup_t[rc * P:(rc + 1) * P, :]
                        )
                    elif dt == 2:
                        nc.sync.dma_start(ones_t[:], ones_in[:, :])
                        nc.sync.dma_start(
                            bkn_t[:], bkn_v.ap().rearrange("(h p) -> p h", p=P)
                        )
                        nc.sync.dma_start(bvv_bc[:], _bcast_rows(bvv_v, P, H * DV))
                    elif dt == 6:
                        nc.sync.dma_start(
                            bqn_t[:], bqn_v.ap().rearrange("(h p) -> p h", p=P)
                        )
                        nc.sync.dma_start(
                            bqr_t[:], bqr_v.ap().rearrange("(a p) -> p a", p=P)
                        )
                        nc.sync.dma_start(cq_t[:], cq_tab[:, :])
                        nc.sync.dma_start(sq_t[:], sq_tab[:, :])

                for ktp in range(2):
                    pm = [
                        psB.tile([P, 2, 512], F32, tag=f"pmix{i}", bufs=2,
                                 name=f"pm{i}")
                        for i in range(2)
                    ]
                    for a in range(4):
                        hk8 = hsq_all[:, ktp, 4 * a:4 * (a + 1), :, :]
                        for di in range(4):
                            dt = 4 * a + di
                            st = (dt == 0)
                            sp = (dt == NDT - 1)
                            for i in range(2):
                                nc.tensor.matmul(
                                    pm[i][:, 0, 0:288], hk8[:, di, i, :],
                                    wkv_all[:, dt, 0:288], start=st, stop=sp,
                                )
                                nc.tensor.matmul(
                                    pm[i][:, 1, 0:288], hk8[:, di, i, :],
                                    wkv_all[:, dt, 288:576], start=st, stop=sp,
                                )
                            if ktp == 0:
                                _stream_weights(dt)
                    for i in range(2):
                        kt = 2 * ktp + i
                        kvmix = mixp.tile([P, R + DR], F32, tag="kvmix")
                        nc.scalar.copy(kvmix[:, 0:288], pm[i][:, 0, 0:288])
                        nc.scalar.copy(kvmix[:, 288:576], pm[i][:, 1, 0:288])

                        stats = lnp.tile([P, 6], F32, tag="stats")
                        nc.vector.bn_stats(stats[:], kvmix[:, 0:R])
                        mv = lnp.tile([P, 2], F32, tag="mv")
                        nc.vector.bn_aggr(mv[:], stats[:])
                        rstd = lnp.tile([P, 1], F32, tag="rstd")
                        nc.scalar.activation(
                            rstd[:], mv[:, 1:2], AF.Sqrt, bias=eps_t[:]
                        )
                        nc.vector.reciprocal(rstd[:], rstd[:])
                        nc.vector.tensor_scalar(
                            lnf_loc[:, kt, :], kvmix[:, 0:R], mv[:, 0:1],
                            rstd[:], op0=OP.subtract, op1=OP.mult,
                        )

                        # RoPE, rotated pairs duplicated to cols 64:128
                        t1 = lnp.tile([P, DR // 2], F32, tag="t1")
                        t2 = lnp.tile([P, DR // 2], F32, tag="t2")
                        x1 = kvmix[:, R:R + 32]
                        x2 = kvmix[:, R + 32:R + 64]
                        kro = kro_loc[:, kt, :]
                        nc.vector.tensor_tensor(t2[:], x1, ck_t[:, kt, :], OP.mult)
                        nc.vector.tensor_tensor(t1[:], x2, sk_t[:, kt, :], OP.mult)
                        nc.vector.tensor_tensor(kro[:, 0:32], t2[:], t1[:], OP.subtract)
                        nc.vector.tensor_tensor(kro[:, 64:96], t2[:], t1[:], OP.subtract)
                        nc.vector.tensor_tensor(t2[:], x1, sk_t[:, kt, :], OP.mult)
                        nc.vector.tensor_tensor(t1[:], x2, ck_t[:, kt, :], OP.mult)
                        nc.vector.tensor_tensor(kro[:, 32:64], t2[:], t1[:], OP.add)
                        nc.vector.tensor_tensor(kro[:, 96:128], t2[:], t1[:], OP.add)

            # ====== segment B2: transposes, local k_nope / v_lat, AllGathers ======
            with ExitStack() as pB2:
                klocp = pB2.enter_context(tc.tile_pool(name="klocp", bufs=1))
                psC = pB2.enter_context(tc.tile_pool(name="psC", bufs=1, space="PSUM"))

                kfl = klocp.tile([P, 5, CH], BF16)
                knl = klocp.tile([P, H, CH], BF16)
                vll = klocp.tile([P, NKTC, H * DV], BF16)

                for kt in range(NKTC):
                    pt = psC.tile([P, 5, P], BF16, tag="ptr", bufs=2)
                    for j in range(4):
                        nc.tensor.transpose(
                            pt[:, j, :],
                            lnf_loc[:, kt, j * P:(j + 1) * P], ident[:],
                        )
                    nc.tensor.transpose(pt[:, 4, :], kro_loc[:, kt, :], ident[:])
                    dst = kfl[:, 0:5, kt * P:(kt + 1) * P]
                    if kt % 2 == 0:
                        nc.vector.tensor_copy(dst, pt[:])
                    else:
                        nc.scalar.copy(dst, pt[:])

                # rope plane: evict + AllGather + reload (tiny, lands early)
                nc.sync.dma_start(rp_in.ap(), kfl[:, 4, :])
                nc.gpsimd.collective_compute(
                    "AllGather", OP.bypass, replica_groups=RG,
                    ins=[rp_in.ap().opt()], outs=[rp_out.ap().opt()],
                )
                nc.sync.dma_start(
                    ropeT[:], rp_out.ap().rearrange("(r p) c -> p r c", p=P)
                )

                # local k_nope: all 16 heads x own 512 keys
                for h in range(H):
                    pk = psC.tile([P, CH], F32, tag="pk", bufs=2)
                    for rc in range(4):
                        nc.tensor.matmul(
                            pk[:], kupT[:, rc, h * P:(h + 1) * P],
                            kfl[:, rc, :],
                            start=(rc == 0), stop=(rc == 3),
                        )
                    nc.scalar.add(knl[:, h, :], pk[:], bkn_t[:, h:h + 1])
                nc.sync.dma_start(
                    kn_in.ap().rearrange("(h p) c -> p h c", p=P), knl[:]
                )
                nc.gpsimd.collective_compute(
                    "AllGather", OP.bypass, replica_groups=RG,
                    ins=[kn_in.ap().opt()], outs=[kn_out.ap().opt()],
                )

                # local v_lat: all 4 groups x own 512 keys
                for kt in range(NKTC):
                    for g in range(G):
                        pvv = psC.tile([P, CH], F32, tag="pvv", bufs=2)
                        for rc in range(4):
                            nc.tensor.matmul(
                                pvv[:], kfl[:, rc, kt * P:(kt + 1) * P],
                                vupT[:, rc, g * 512:(g + 1) * 512],
                                start=(rc == 0), stop=(rc == 3),
                            )
                        nc.vector.tensor_tensor(
                            vll[:, kt, g * 512:(g + 1) * 512], pvv[:],
                            bvv_bc[:, g * 512:(g + 1) * 512], OP.add,
                        )
                nc.sync.dma_start(
                    vl_in.ap().rearrange("(t p) c -> p t c", p=P), vll[:]
                )
                nc.gpsimd.collective_compute(
                    "AllGather", OP.bypass, replica_groups=RG,
                    ins=[vl_in.ap().opt()], outs=[vl_out.ap().opt()],
                )

            # ================= segment A: q-mix matmuls + LN =================
            with ExitStack() as pA:
                wqap = pA.enter_context(tc.tile_pool(name="wqap", bufs=4))
                mixp = pA.enter_context(tc.tile_pool(name="mixp", bufs=1))
                lnp = pA.enter_context(tc.tile_pool(name="lnp", bufs=2))
                psA = pA.enter_context(tc.tile_pool(name="psA", bufs=1, space="PSUM"))

                qmix_all = mixp.tile([P, NQT, LAT], BF16)
                for j in range(3):
                    pqj = psA.tile([P, NQT, 512], F32, tag="pq", bufs=2)
                    for a in range(4):
                        wqa_c = wqap.tile([P, 4, 512], BF16, tag="wqa")
                        nc.sync.dma_start(
                            wqa_c[:],
                            wqa_t[a * 512:(a + 1) * 512,
                                  j * 512:(j + 1) * 512].rearrange(
                                "(t p) c -> p t c", p=P
                            ),
                        )
                        for i in range(4):
                            dt = 4 * a + i
                            for qt in range(NQT):
                                nc.tensor.matmul(
                                    pqj[:, qt, :],
                                    hsq_all[:, qt // 2, dt, qt % 2, :],
                                    wqa_c[:, i, :],
                                    start=(dt == 0), stop=(dt == NDT - 1),
                                )
                    for qt in range(NQT):
                        nc.vector.tensor_copy(
                            qmix_all[:, qt, j * 512:(j + 1) * 512], pqj[:, qt, :]
                        )

                for qt in range(NQT):
                    statsq = lnp.tile([P, 3, 6], F32, tag="statsq")
                    for j in range(3):
                        nc.vector.bn_stats(
                            statsq[:, j, :], qmix_all[:, qt, j * 512:(j + 1) * 512]
                        )
                    mvq = lnp.tile([P, 2], F32, tag="mv")
                    nc.vector.bn_aggr(mvq[:], statsq[:])
                    rstdq = lnp.tile([P, 1], F32, tag="rstd")
                    nc.scalar.activation(
                        rstdq[:], mvq[:, 1:2], AF.Sqrt, bias=eps_t[:]
                    )
                    nc.vector.reciprocal(rstdq[:], rstdq[:])
                    nc.vector.tensor_scalar(
                        qln_all[:, qt, :], qmix_all[:, qt, :], mvq[:, 0:1],
                        rstdq[:], op0=OP.subtract, op1=OP.mult,
                    )

            # ============ segment C: batched q latent transposes ============
            with ExitStack() as pC:
                psC2 = pC.enter_context(tc.tile_pool(name="psC2", bufs=1, space="PSUM"))
                ev = 0
                for qt in range(NQT):
                    for half in range(2):
                        ptq = psC2.tile([P, 6, P], BF16, tag="ptr", bufs=2)
                        for i in range(6):
                            lt = half * 6 + i
                            nc.tensor.transpose(
                                ptq[:, i, :],
                                qln_all[:, qt, lt * P:(lt + 1) * P], ident[:],
                            )
                        dst = qlat_t[:, half * 6:(half + 1) * 6,
                                     qt * P:(qt + 1) * P]
                        if ev % 2 == 0:
                            nc.vector.tensor_copy(dst, ptq[:])
                        else:
                            nc.scalar.copy(dst, ptq[:])
                        ev += 1

            # ===== phase 1.5: q_rope (4 groups) + q_nope (16 heads) staged =====
            with ExitStack() as p15:
                wqs = p15.enter_context(tc.tile_pool(name="wqs", bufs=2))
                qwork = p15.enter_context(tc.tile_pool(name="qwork", bufs=2))
                ps15 = p15.enter_context(tc.tile_pool(name="ps15", bufs=1, space="PSUM"))

                for g in range(G):
                    qraw = qwork.tile([P, 2, CH], F32, tag="qraw")
                    for half in range(2):
                        wrc = wqs.tile([P, NLT, P], BF16, tag="wq")
                        col0 = half * 512 + g * P
                        nc.sync.dma_start(
                            wrc[:],
                            wqr_t[:, col0:col0 + P].rearrange(
                                "(t p) c -> p t c", p=P
                            ),
                        )
                        pr = ps15.tile([P, 512], F32, tag="proj", bufs=2)
                        for lt in range(NLT):
                            nc.tensor.matmul(
                                pr[:], wrc[:, lt, :], qlat_t[:, lt, :],
                                start=(lt == 0), stop=(lt == NLT - 1),
                            )
                        nc.scalar.add(
                            qraw[:, half, :], pr[:],
                            bqr_t[:, half * 4 + g:half * 4 + g + 1],
                        )
                    tm = qwork.tile([P, CH], F32, tag="tm")
                    tn = qwork.tile([P, CH], F32, tag="tn")
                    x1, x2 = qraw[:, 0, :], qraw[:, 1, :]
                    nc.vector.tensor_tensor(tm[:], x2, sq_t[:], OP.mult)
                    nc.vector.tensor_tensor(tn[:], x1, cq_t[:], OP.mult)
                    nc.vector.tensor_tensor(qro_all[:, g, 0, :], tn[:], tm[:], OP.subtract)
                    nc.vector.tensor_tensor(tm[:], x2, cq_t[:], OP.mult)
                    nc.vector.tensor_tensor(tn[:], x1, sq_t[:], OP.mult)
                    nc.vector.tensor_tensor(qro_all[:, g, 1, :], tn[:], tm[:], OP.add)

                for h in range(H):
                    wb = wqs.tile([P, NLT, P], BF16, tag="wq")
                    nc.sync.dma_start(
                        wb[:],
                        wqb_t[:, h * P:(h + 1) * P].rearrange(
                            "(t p) c -> p t c", p=P
                        ),
                    )
                    pn = ps15.tile([P, 512], F32, tag="proj", bufs=2)
                    for lt in range(NLT):
                        nc.tensor.matmul(
                            pn[:], wb[:, lt, :], qlat_t[:, lt, :],
                            start=(lt == 0), stop=(lt == NLT - 1),
                        )
                    nc.scalar.add(qnope_all[:, h, :], pn[:], bqn_t[:, h:h + 1])

        # ====================== phase 2: attention head loop ======================
        attp = octx.enter_context(tc.tile_pool(name="attp", bufs=1))
        avT = attp.tile([P, H, CH], BF16)

        wo_pre = []
        with ExitStack() as p2:
            hwork = p2.enter_context(tc.tile_pool(name="hwork", bufs=1))
            gwork = p2.enter_context(tc.tile_pool(name="gwork", bufs=2))
            probs_p = p2.enter_context(tc.tile_pool(name="probs_p", bufs=2))
            foldp = p2.enter_context(tc.tile_pool(name="foldp", bufs=3))
            ps2 = p2.enter_context(tc.tile_pool(name="ps2", bufs=1, space="PSUM"))

            def load_knope(h):
                t = hwork.tile([P, S], BF16, tag="knopeT", bufs=3)
                nc.sync.dma_start(
                    t[:],
                    kn_out.ap().rearrange("(r h p) c -> p h r c", r=G, h=H)[:, h],
                )
                return t

            def load_vlat(g):
                t = gwork.tile([P, NKT, CH], BF16, tag="vlatq", bufs=2)
                nc.sync.dma_start(
                    t[:],
                    vl_out.ap().rearrange(
                        "(r t p) (g c) -> p g r t c", r=G, t=NKTC, g=G
                    )[:, g],
                )
                return t

            kn_tiles = [load_knope(0), load_knope(1)]
            vl_tiles = [load_vlat(0)]

            for h in range(H):
                g, m = divmod(h, 4)
                knopeT = kn_tiles[h]
                vlatq = vl_tiles[g]
                if h + 2 < H:
                    kn_tiles.append(load_knope(h + 2))
                if m == 3 and g + 1 < G:
                    vl_tiles.append(load_vlat(g + 1))

                if h == H - 1:
                    for i in range(3):
                        wo = wop.tile([P, 512], BF16, tag="wo")
                        nc.sync.dma_start(wo[:], wo_t[i * P:(i + 1) * P, 0:512])
                        wo_pre.append(wo)

                qnope = qnope_all[:, h, :]
                qro = qro_all[:, g, :, :]
                qropeT = hwork.tile([P, CH], BF16, tag="qropeT", bufs=2)
                nc.sync.dma_start(qropeT[0:32, :], qro[m * 32:(m + 1) * 32, 0, :])
                nc.sync.dma_start(qropeT[32:64, :], qro[m * 32:(m + 1) * 32, 1, :])
                nc.sync.dma_start(qropeT[64:96, :], qro[m * 32:(m + 1) * 32, 0, :])
                nc.sync.dma_start(qropeT[96:128, :], qro[m * 32:(m + 1) * 32, 1, :])

                probs = probs_p.tile([P, NKT, CH], BF16, tag="probs")
                folds = []
                quads = []
                octs = []
                pv = ps2.tile([P, 512], F32, tag="attn", bufs=1)
                pd = ps2.tile([P, 512], F32, tag="den", bufs=1)
                for p in range(NKT // 2):
                    kt, kt1 = 2 * p, 2 * p + 1
                    sc = ps2.tile([P, 2, 512], F32, tag="scores", bufs=2)
                    nc.tensor.matmul(
                        sc[:, 0, :], knopeT[:, kt * P:(kt + 1) * P], qnope,
                        start=True, stop=False,
                    )
                    nc.tensor.matmul(
                        sc[:, 1, :], knopeT[:, kt1 * P:(kt1 + 1) * P], qnope,
                        start=True, stop=False,
                    )
                    nc.tensor.matmul(
                        sc[:, 0, :], ropeT[0:DR, kt * P:(kt + 1) * P],
                        qropeT[0:DR, :], start=False, stop=True,
                    )
                    nc.tensor.matmul(
                        sc[:, 1, :], ropeT[DR:P, kt1 * P:(kt1 + 1) * P],
                        qropeT[DR:P, :], start=False, stop=True,
                        tile_position=(DR, 0),
                    )
                    nc.scalar.activation(probs[:, kt:kt + 2, :], sc[:], AF.Exp)
                    ft = foldp.tile([P, CH], F32R, tag="fold")
                    nc.vector.tensor_tensor(
                        ft[:], probs[:, kt, :], probs[:, kt1, :], OP.add
                    )
                    folds.append(ft)
                    if p % 2 == 1:
                        fq = foldp.tile([P, CH], F32R, tag="foldq")
                        nc.vector.tensor_tensor(
                            fq[:], folds[p - 1][:], folds[p][:], OP.add
                        )
                        quads.append(fq)
                    if p % 4 == 3:
                        fo = foldp.tile([P, CH], F32R, tag="foldo")
                        nc.vector.tensor_tensor(
                            fo[:], quads[-2][:], quads[-1][:], OP.add
                        )
                        octs.append(fo)
                    if p >= 1:
                        nc.tensor.matmul(
                            pv[:], vlatq[:, kt - 2, m * P:(m + 1) * P],
                            probs[:, kt - 2, :], start=(p == 1), stop=False,
                        )
                        nc.tensor.matmul(
                            pv[:], vlatq[:, kt - 1, m * P:(m + 1) * P],
                            probs[:, kt - 1, :], start=False, stop=False,
                        )
                    if p == 5:
                        nc.tensor.matmul(
                            pd[:], ones_t[:], octs[0][:],
                            start=True, stop=False,
                        )
                nc.tensor.matmul(
                    pv[:], vlatq[:, NKT - 2, m * P:(m + 1) * P],
                    probs[:, NKT - 2, :], start=False, stop=False,
                )
                nc.tensor.matmul(
                    pv[:], vlatq[:, NKT - 1, m * P:(m + 1) * P],
                    probs[:, NKT - 1, :], start=False, stop=True,
                )
                nc.tensor.matmul(
                    pd[:], ones_t[:], octs[1][:], start=False, stop=True,
                )
                recip = hwork.tile([P, CH], F32, tag="recip", bufs=2)
                nc.vector.reciprocal_approx_fast(recip[:], pd[:])
                nc.vector.tensor_tensor(avT[:, h, :], pv[:], recip[:], OP.mult)

        # ================== phase 3: o_proj in quarter passes ==================
        with ExitStack() as p3:
            outp = p3.enter_context(tc.tile_pool(name="outp", bufs=4))
            ps3 = p3.enter_context(tc.tile_pool(name="ps3", bufs=1, space="PSUM"))

            pre = wo_pre
            for quarter in range(4):
                po = ps3.tile([P, NQT, 512], F32, tag="po", bufs=2)
                for kt in range(H):
                    if kt < len(pre):
                        wo = pre[kt]
                    else:
                        wo = wop.tile([P, 512], BF16, tag="wo")
                        nc.sync.dma_start(
                            wo[:],
                            wo_t[kt * P:(kt + 1) * P,
                                 quarter * 512:(quarter + 1) * 512],
                        )
                    for qc in range(NQT):
                        nc.tensor.matmul(
                            po[:, qc, :],
                            avT[:, kt, qc * P:(qc + 1) * P],
                            wo[:],
                            start=(kt == 0), stop=(kt == H - 1),
                        )
                pre = []
                if quarter < 3:
                    for i in range(2):
                        wo = wop.tile([P, 512], BF16, tag="wo")
                        nc.sync.dma_start(
                            wo[:],
                            wo_t[i * P:(i + 1) * P,
                                 (quarter + 1) * 512:(quarter + 2) * 512],
                        )
                        pre.append(wo)
                for qc in range(NQT):
                    ot = outp.tile([P, 512], F32, tag="ot")
                    if qc % 2 == 0:
                        nc.vector.tensor_copy(ot[:], po[:, qc, :])
                    else:
                        nc.scalar.copy(ot[:], po[:, qc, :])
                    nc.sync.dma_start(
                        out_c[
                            qc * P:(qc + 1) * P,
                            quarter * 512:(quarter + 1) * 512,
                        ],
                        ot[:],
                    )

    nc.compile()
    return nc


_NC_CACHE = None


def _get_nc():
    global _NC_CACHE
    if _NC_CACHE is None:
        _NC_CACHE = build_nc()
    return _NC_CACHE


def _prep_in_maps(inputs):
    hidden = np.asarray(inputs["hidden_states"], dtype=np.float32)
    w_qa = np.asarray(inputs["w_qa"], dtype=np.float32)
    ln_qa_g = np.asarray(inputs["ln_qa_g"], dtype=np.float32)
    ln_qa_b = np.asarray(inputs["ln_qa_b"], dtype=np.float32)
    w_qb = np.asarray(inputs["w_qb"], dtype=np.float32)
    w_qrope = np.asarray(inputs["w_qrope"], dtype=np.float32)
    w_kva = np.asarray(inputs["w_kva"], dtype=np.float32)
    ln_kva_g = np.asarray(inputs["ln_kva_g"], dtype=np.float32)
    ln_kva_b = np.asarray(inputs["ln_kva_b"], dtype=np.float32)
    w_kvb = np.asarray(inputs["w_kvb"], dtype=np.float32)
    w_o = np.asarray(inputs["w_o"], dtype=np.float32)
    pos = np.asarray(inputs["position_ids"]).astype(np.int64)

    bf = bfloat16
    hidden_b = hidden.astype(bf)
    hst_all = [
        hidden_b[b].T.reshape(NDT, P, NKT // 2, 2, P).transpose(2, 0, 3, 1, 4)
        for b in range(B)
    ]
    wqa_t = np.ascontiguousarray(w_qa.T.astype(bf))
    # LN gamma folded into q up-projections; beta becomes an output bias:
    # q_nope = (ln0*g + b) @ w_qb.T = ln0 @ (w_qb*g).T + w_qb @ b
    wqb_g = w_qb * ln_qa_g[None, :]
    bqn = (w_qb @ ln_qa_b).astype(np.float32)
    wqb_t = np.ascontiguousarray(wqb_g.T.astype(bf))
    wqr_s = SCALE * w_qrope
    bqr_full = (wqr_s @ ln_qa_b).astype(np.float32)
    wqr_g = (wqr_s * ln_qa_g[None, :]).T
    wqr_t = np.ascontiguousarray(
        wqr_g.reshape(LAT, H, 2, DR // 2).transpose(0, 2, 1, 3)
        .reshape(LAT, H * DR).astype(bf)
    )
    bqr_perm = np.ascontiguousarray(
        bqr_full.reshape(H, 2, DR // 2).transpose(1, 0, 2).reshape(H * DR)
    )
    wkva_t = np.ascontiguousarray(w_kva.T.astype(bf))
    kup = (SCALE * w_kvb[: H * DN]).reshape(H, DN, R)
    bkn = (kup @ ln_kva_b).reshape(H * DN).astype(np.float32)
    kup_g = kup * ln_kva_g[None, None, :]
    kup_t = np.ascontiguousarray(
        kup_g.transpose(2, 0, 1).reshape(R, H * DN).astype(bf)
    )
    vup = w_kvb[H * DN:].reshape(H, DV, R)
    bvv = (vup @ ln_kva_b).reshape(H * DV).astype(np.float32)
    vup_g = vup * ln_kva_g[None, None, :]
    vup_t = np.ascontiguousarray(
        vup_g.transpose(2, 0, 1).reshape(R, H * DV).astype(bf)
    )
    wo_t = np.ascontiguousarray(w_o.T.astype(bf))
    ones_in = np.ones((P, P), dtype=np.float32)

    inv_freq = 1.0 / (10000.0 ** (np.arange(0, DR, 2, dtype=np.float64) / DR))
    ang = pos[:, None].astype(np.float64) * inv_freq[None, :]
    cosf = np.ascontiguousarray(np.cos(ang).astype(np.float32))
    sinf = np.ascontiguousarray(np.sin(ang).astype(np.float32))

    in_maps = []
    for c in range(N_CORES):
        b, ch = divmod(c, NQT)
        qs = ch * CH
        cq = np.ascontiguousarray(np.tile(cosf[qs:qs + CH, :].T, (NQT, 1)))
        sq = np.ascontiguousarray(np.tile(sinf[qs:qs + CH, :].T, (NQT, 1)))
        myp = [2 * ch, 2 * ch + 1]
        in_maps.append({
            "hsqt": np.ascontiguousarray(hst_all[b][myp]),
            "wqa_t": wqa_t,
            "wqb_t": wqb_t,
            "wqr_t": wqr_t,
            "wkva_t": wkva_t,
            "kup_t": kup_t,
            "vup_t": vup_t,
            "wo_t": wo_t,
            "bqn_v": bqn,
            "bqr_v": bqr_perm,
            "bkn_v": bkn,
            "bvv_v": bvv,
            "ones_in": ones_in,
            "ck_tab": np.ascontiguousarray(cosf[qs:qs + CH]),
            "sk_tab": np.ascontiguousarray(sinf[qs:qs + CH]),
            "cq_tab": cq,
            "sq_tab": sq,
        })
    return in_maps


def kernel(**inputs) -> np.ndarray:
    nc = _get_nc()
    in_maps = _prep_in_maps(inputs)
    res = run_bass_kernel_spmd(nc, in_maps, core_ids=list(range(N_CORES)))
    out = np.empty((B, S, D), dtype=np.float32)
    for c in range(N_CORES):
        b, ch = divmod(c, NQT)
        out[b, ch * CH:(ch + 1) * CH, :] = res.results[c]["out_c"]
    return out


# revision 8
# speedup vs baseline: 1.0503x; 1.0503x over previous
"""MLA (multi-head latent attention) prefill kernel for 8 Trainium2 NeuronCores.

Sharding: data-parallel over (batch, query-chunk) for the q path, attention
and o_proj. The KV path is split between recompute and collectives, sized to
the measured ~55 GB/s 4-rank AllGather rate (replica groups [[0,1,2,3],
[4,5,6,7]], one per batch):
 - kv-mix runs only over the core's own 512 keys (the same hidden rows as its
   q chunk, so no cross-chunk hidden-state DMAs at all); the resulting
   latent+rope planes (0.65MB) are AllGathered into the full kfull (AG1).
 - k_nope for heads 0-7 and v_lat for head-groups 0-1 are recomputed per-core
   from kfull inside the head loop (baseline style).
 - k_nope for heads 8-15 and v_lat for groups 2-3 are computed once per core
   over its own 512 keys, AllGathered (AG2/AG3, 4MB each) while the PE chews
   through the q path, and DMA-loaded just-in-time in the head loop.
Collectives run on TOPSP/SDMA silicon and overlap with PE compute; AG2/AG3
have >100us of slack before their consumers. Per-core matmul work drops from
~37.6 GF to ~30 GF with zero PE stalls by construction.

Structure:
 - Segment B': kv-mix over own keys, LN + RoPE, transposes -> kfl (5 planes),
   evict + AG1; local k_nope[h8..15] / v_lat[g2..3] shares -> AG2 / AG3.
 - Segment A: q-mix matmuls + LayerNorm (unchanged from baseline).
 - Segment C: q latent transposes (batched, 6/bank).
 - Phase 1.5: q_rope for all 4 head groups + q_nope for all 16 heads staged
   in SBUF; the gathered kfull is DMA-loaded here (after AG1).
 - Phase 2 head loop: per head, scores contract 128+64 channels; softmax
   denominator via pair->quad->oct folds + two ones-matmuls; attn @ v_lat.
 - Phase 3: o_proj in 4 quarter passes with double-buffered PSUM.
"""

import math
from contextlib import ExitStack

import numpy as np
from ml_dtypes import bfloat16

import concourse.bass as bass
import concourse.tile as tile
from concourse import bacc, mybir
from concourse.bass_utils import run_bass_kernel_spmd
from concourse.masks import make_identity

F32 = mybir.dt.float32
F32R = mybir.dt.float32r
BF16 = mybir.dt.bfloat16
AF = mybir.ActivationFunctionType
OP = mybir.AluOpType

B, S, D = 2, 2048, 2048
H = 16
LAT = 1536
R = 512
DN, DR, DV = 128, 64, 128
EPS = 1e-5
SCALE = 1.0 / math.sqrt(DN + DR)

P = 128
CH = 512
NQT = CH // P      # 4 q tiles per chunk
NKT = S // P       # 16 key tiles total
NKTC = CH // P     # 4 key tiles in own chunk
NDT = D // P
NLT = LAT // P

N_CORES = 8
G = 4              # ranks per replica group (one batch)
RG = [[0, 1, 2, 3], [4, 5, 6, 7]]
H_GATH = 8         # heads [H_GATH..H) use gathered k_nope
G_GATH = 2         # groups [G_GATH..4) use gathered v_lat
NHG = H - H_GATH
NGG = 4 - G_GATH


def _bcast_rows(t, n, length):
    return bass.AP(tensor=t, offset=0, ap=[[0, n], [1, length]])


def build_nc():
    nc = bacc.Bacc(None, target_bir_lowering=False, num_devices=N_CORES)

    hsqt = nc.dram_tensor("hsqt", [2, NDT, 2, P, P], BF16, kind="ExternalInput")
    wqa_t = nc.dram_tensor("wqa_t", [D, LAT], BF16, kind="ExternalInput")
    wqb_t = nc.dram_tensor("wqb_t", [LAT, H * DN], BF16, kind="ExternalInput")
    wqr_t = nc.dram_tensor("wqr_t", [LAT, H * DR], BF16, kind="ExternalInput")
    wkva_t = nc.dram_tensor("wkva_t", [D, R + DR], BF16, kind="ExternalInput")
    kup_t = nc.dram_tensor("kup_t", [R, H * DN], BF16, kind="ExternalInput")
    vup_t = nc.dram_tensor("vup_t", [R, H * DV], BF16, kind="ExternalInput")
    wo_t = nc.dram_tensor("wo_t", [H * DV, D], BF16, kind="ExternalInput")
    bqn_v = nc.dram_tensor("bqn_v", [H * DN], F32, kind="ExternalInput")
    bqr_v = nc.dram_tensor("bqr_v", [H * DR], F32, kind="ExternalInput")
    bkn_v = nc.dram_tensor("bkn_v", [H * DN], F32, kind="ExternalInput")
    bvv_v = nc.dram_tensor("bvv_v", [H * DV], F32, kind="ExternalInput")
    ones_in = nc.dram_tensor("ones_in", [P, P], F32R, kind="ExternalInput")
    ck_tab = nc.dram_tensor("ck_tab", [CH, DR // 2], F32, kind="ExternalInput")
    sk_tab = nc.dram_tensor("sk_tab", [CH, DR // 2], F32, kind="ExternalInput")
    cq_tab = nc.dram_tensor("cq_tab", [P, CH], F32, kind="ExternalInput")
    sq_tab = nc.dram_tensor("sq_tab", [P, CH], F32, kind="ExternalInput")
    out_c = nc.dram_tensor("out_c", [CH, D], F32, kind="ExternalOutput")

    # collective bounce buffers (internal DRAM)
    kv_in = nc.dram_tensor("kv_in", [5 * P, CH], BF16)
    kv_out = nc.dram_tensor("kv_out", [G * 5 * P, CH], BF16)
    kn_in = nc.dram_tensor("kn_in", [NHG * P, CH], BF16)
    kn_out = nc.dram_tensor("kn_out", [G * NHG * P, CH], BF16)
    vl_in = nc.dram_tensor("vl_in", [CH, NGG * CH], BF16)
    vl_out = nc.dram_tensor("vl_out", [G * CH, NGG * CH], BF16)

    with tile.TileContext(nc) as tc, ExitStack() as octx:
        res = octx.enter_context(tc.tile_pool(name="res", bufs=1))
        kfull = res.tile([P, 5, S], BF16)   # gathered latent^T (4) + rope^T (1)
        kupT = res.tile([P, 4, H * DN], BF16)
        vupT = res.tile([P, 4, H * DV], BF16)

        consts = octx.enter_context(tc.tile_pool(name="consts", bufs=1))
        ident = consts.tile([P, P], BF16)
        make_identity(nc, ident)
        ones_t = consts.tile([P, P], F32R)
        eps_t = consts.tile([P, 1], F32)
        nc.vector.memset(eps_t, EPS)
        cq_t = consts.tile([P, CH], F32)
        sq_t = consts.tile([P, CH], F32)
        bqn_t = consts.tile([P, H], F32)
        bqr_t = consts.tile([P, 8], F32)
        bkn_t = consts.tile([P, H], F32)
        bvv_bc = consts.tile([P, H * DV], F32)

        qstage = octx.enter_context(tc.tile_pool(name="qstage", bufs=1))
        qro_all = qstage.tile([P, G, 2, CH], BF16)
        qnope_all = qstage.tile([P, H, CH], BF16)

        wop = octx.enter_context(tc.tile_pool(name="wop", bufs=6))

        with ExitStack() as p1all:
            lnsp = p1all.enter_context(tc.tile_pool(name="lnsp", bufs=1))
            qproj = p1all.enter_context(tc.tile_pool(name="qproj", bufs=1))
            qlat_t = qproj.tile([P, NLT, CH], BF16)

            qln_all = lnsp.tile([P, NQT, LAT], BF16)
            lnf_loc = lnsp.tile([P, NKTC, R], BF16)
            kro_loc = lnsp.tile([P, NKTC, P], BF16)

            p1ab = p1all.enter_context(ExitStack())
            hsqp = p1ab.enter_context(tc.tile_pool(name="hsqp", bufs=1))
            wkvp = p1ab.enter_context(tc.tile_pool(name="wkvp", bufs=1))
            hsq_all = hsqp.tile([P, 2, NDT, 2, P], BF16)
            wkv_all = wkvp.tile([P, NDT, R + DR], BF16)
            ck_t = wkvp.tile([P, NKTC, DR // 2], F32)
            sk_t = wkvp.tile([P, NKTC, DR // 2], F32)

            # =============== segment B': kv-mix on own 512 keys ===============
            with ExitStack() as pB:
                mixp = pB.enter_context(tc.tile_pool(name="mixp", bufs=3))
                lnp = pB.enter_context(tc.tile_pool(name="lnp", bufs=2))
                psB = pB.enter_context(tc.tile_pool(name="psB", bufs=1, space="PSUM"))

                # interleave kv weights with own hidden-state tiles so the
                # first matmul can start after ~1.1MB of DMA
                for a in range(4):
                    nc.sync.dma_start(
                        wkv_all[:, 4 * a:4 * (a + 1), :],
                        wkva_t[512 * a:512 * (a + 1), :].rearrange(
                            "(t p) c -> p t c", p=P
                        ),
                    )
                    for pair in range(2):
                        nc.sync.dma_start(
                            hsq_all[:, pair, 4 * a:4 * (a + 1), :, :],
                            hsqt[pair, 4 * a:4 * (a + 1)].rearrange(
                                "d k p c -> p d k c"
                            ),
                        )
                nc.sync.dma_start(
                    ck_t[:], ck_tab.ap().rearrange("(t p) j -> p t j", p=P)
                )
                nc.sync.dma_start(
                    sk_t[:], sk_tab.ap().rearrange("(t p) j -> p t j", p=P)
                )

                # phase-2 weights streamed during kv-mix
                def _stream_weights(dt):
                    if dt % 4 == 1:
                        rc = dt // 4
                        nc.sync.dma_start(
                            kupT[:, rc, :], kup_t[rc * P:(rc + 1) * P, :]
                        )
                    elif dt % 4 == 3:
                        rc = dt // 4
                        nc.sync.dma_start(
                            vupT[:, rc, :], vup_t[rc * P:(rc + 1) * P, :]
                        )
                    elif dt == 2:
                        nc.sync.dma_start(ones_t[:], ones_in[:, :])
                        nc.sync.dma_start(
                            bkn_t[:], bkn_v.ap().rearrange("(h p) -> p h", p=P)
                        )
                        nc.sync.dma_start(bvv_bc[:], _bcast_rows(bvv_v, P, H * DV))
                    elif dt == 6:
                        nc.sync.dma_start(
                            bqn_t[:], bqn_v.ap().rearrange("(h p) -> p h", p=P)
                        )
                        nc.sync.dma_start(
                            bqr_t[:], bqr_v.ap().rearrange("(a p) -> p a", p=P)
                        )
                        nc.sync.dma_start(cq_t[:], cq_tab[:, :])
                        nc.sync.dma_start(sq_t[:], sq_tab[:, :])

                for ktp in range(2):
                    pm = [
                        psB.tile([P, 2, 512], F32, tag=f"pmix{i}", bufs=2,
                                 name=f"pm{i}")
                        for i in range(2)
                    ]
                    for a in range(4):
                        hk8 = hsq_all[:, ktp, 4 * a:4 * (a + 1), :, :]
                        for di in range(4):
                            dt = 4 * a + di
                            st = (dt == 0)
                            sp = (dt == NDT - 1)
                            for i in range(2):
                                nc.tensor.matmul(
                                    pm[i][:, 0, 0:288], hk8[:, di, i, :],
                                    wkv_all[:, dt, 0:288], start=st, stop=sp,
                                )
                                nc.tensor.matmul(
                                    pm[i][:, 1, 0:288], hk8[:, di, i, :],
                                    wkv_all[:, dt, 288:576], start=st, stop=sp,
                                )
                            if ktp == 0:
                                _stream_weights(dt)
                    for i in range(2):
                        kt = 2 * ktp + i
                        kvmix = mixp.tile([P, R + DR], F32, tag="kvmix")
                        nc.scalar.copy(kvmix[:, 0:288], pm[i][:, 0, 0:288])
                        nc.scalar.copy(kvmix[:, 288:576], pm[i][:, 1, 0:288])

                        stats = lnp.tile([P, 6], F32, tag="stats")
                        nc.vector.bn_stats(stats[:], kvmix[:, 0:R])
                        mv = lnp.tile([P, 2], F32, tag="mv")
                        nc.vector.bn_aggr(mv[:], stats[:])
                        rstd = lnp.tile([P, 1], F32, tag="rstd")
                        nc.scalar.activation(
                            rstd[:], mv[:, 1:2], AF.Sqrt, bias=eps_t[:]
                        )
                        nc.vector.reciprocal(rstd[:], rstd[:])
                        nc.vector.tensor_scalar(
                            lnf_loc[:, kt, :], kvmix[:, 0:R], mv[:, 0:1],
                            rstd[:], op0=OP.subtract, op1=OP.mult,
                        )

                        # RoPE, rotated pairs duplicated to cols 64:128
                        t1 = lnp.tile([P, DR // 2], F32, tag="t1")
                        t2 = lnp.tile([P, DR // 2], F32, tag="t2")
                        x1 = kvmix[:, R:R + 32]
                        x2 = kvmix[:, R + 32:R + 64]
                        kro = kro_loc[:, kt, :]
                        nc.vector.tensor_tensor(t2[:], x1, ck_t[:, kt, :], OP.mult)
                        nc.vector.tensor_tensor(t1[:], x2, sk_t[:, kt, :], OP.mult)
                        nc.vector.tensor_tensor(kro[:, 0:32], t2[:], t1[:], OP.subtract)
                        nc.vector.tensor_tensor(kro[:, 64:96], t2[:], t1[:], OP.subtract)
                        nc.vector.tensor_tensor(t2[:], x1, sk_t[:, kt, :], OP.mult)
                        nc.vector.tensor_tensor(t1[:], x2, ck_t[:, kt, :], OP.mult)
                        nc.vector.tensor_tensor(kro[:, 32:64], t2[:], t1[:], OP.add)
                        nc.vector.tensor_tensor(kro[:, 96:128], t2[:], t1[:], OP.add)

            # == segment B2: transposes + AG1; gathered-share k_nope / v_lat ==
            with ExitStack() as pB2:
                klocp = pB2.enter_context(tc.tile_pool(name="klocp", bufs=1))
                psC = pB2.enter_context(tc.tile_pool(name="psC", bufs=1, space="PSUM"))

                kfl = klocp.tile([P, 5, CH], BF16)
                kng = klocp.tile([P, NHG, CH], BF16)
                vlg = klocp.tile([P, NKTC, NGG * CH], BF16)

                for kt in range(NKTC):
                    pt = psC.tile([P, 5, P], BF16, tag="ptr", bufs=2)
                    for j in range(4):
                        nc.tensor.transpose(
                            pt[:, j, :],
                            lnf_loc[:, kt, j * P:(j + 1) * P], ident[:],
                        )
                    nc.tensor.transpose(pt[:, 4, :], kro_loc[:, kt, :], ident[:])
                    dst = kfl[:, 0:5, kt * P:(kt + 1) * P]
                    if kt % 2 == 0:
                        nc.vector.tensor_copy(dst, pt[:])
                    else:
                        nc.scalar.copy(dst, pt[:])

                nc.sync.dma_start(
                    kv_in.ap().rearrange("(a p) c -> p a c", p=P), kfl[:]
                )
                nc.gpsimd.collective_compute(
                    "AllGather", OP.bypass, replica_groups=RG,
                    ins=[kv_in.ap().opt()], outs=[kv_out.ap().opt()],
                )

                # gathered-share k_nope: heads 8..15 x own 512 keys
                for j in range(NHG):
                    h = H_GATH + j
                    pk = psC.tile([P, CH], F32, tag="pk", bufs=2)
                    for rc in range(4):
                        nc.tensor.matmul(
                            pk[:], kupT[:, rc, h * P:(h + 1) * P],
                            kfl[:, rc, :],
                            start=(rc == 0), stop=(rc == 3),
                        )
                    nc.scalar.add(kng[:, j, :], pk[:], bkn_t[:, h:h + 1])
                nc.sync.dma_start(
                    kn_in.ap().rearrange("(h p) c -> p h c", p=P), kng[:]
                )
                nc.gpsimd.collective_compute(
                    "AllGather", OP.bypass, replica_groups=RG,
                    ins=[kn_in.ap().opt()], outs=[kn_out.ap().opt()],
                )

                # gathered-share v_lat: groups 2..3 x own 512 keys
                for kt in range(NKTC):
                    for gg in range(NGG):
                        g = G_GATH + gg
                        pvv = psC.tile([P, CH], F32, tag="pvv", bufs=2)
                        for rc in range(4):
                            nc.tensor.matmul(
                                pvv[:], kfl[:, rc, kt * P:(kt + 1) * P],
                                vupT[:, rc, g * 512:(g + 1) * 512],
                                start=(rc == 0), stop=(rc == 3),
                            )
                        nc.vector.tensor_tensor(
                            vlg[:, kt, gg * 512:(gg + 1) * 512], pvv[:],
                            bvv_bc[:, g * 512:(g + 1) * 512], OP.add,
                        )
                nc.sync.dma_start(
                    vl_in.ap().rearrange("(t p) c -> p t c", p=P), vlg[:]
                )
                nc.gpsimd.collective_compute(
                    "AllGather", OP.bypass, replica_groups=RG,
                    ins=[vl_in.ap().opt()], outs=[vl_out.ap().opt()],
                )

            # ================= segment A: q-mix matmuls + LN =================
            with ExitStack() as pA:
                wqap = pA.enter_context(tc.tile_pool(name="wqap", bufs=4))
                mixp = pA.enter_context(tc.tile_pool(name="mixp", bufs=1))
                lnp = pA.enter_context(tc.tile_pool(name="lnp", bufs=2))
                psA = pA.enter_context(tc.tile_pool(name="psA", bufs=1, space="PSUM"))

                qmix_all = mixp.tile([P, NQT, LAT], BF16)
                for j in range(3):
                    pqj = psA.tile([P, NQT, 512], F32, tag="pq", bufs=2)
                    for a in range(4):
                        wqa_c = wqap.tile([P, 4, 512], BF16, tag="wqa")
                        nc.sync.dma_start(
                            wqa_c[:],
                            wqa_t[a * 512:(a + 1) * 512,
                                  j * 512:(j + 1) * 512].rearrange(
                                "(t p) c -> p t c", p=P
                            ),
                        )
                        for i in range(4):
                            dt = 4 * a + i
                            for qt in range(NQT):
                                nc.tensor.matmul(
                                    pqj[:, qt, :],
                                    hsq_all[:, qt // 2, dt, qt % 2, :],
                                    wqa_c[:, i, :],
                                    start=(dt == 0), stop=(dt == NDT - 1),
                                )
                    for qt in range(NQT):
                        nc.vector.tensor_copy(
                            qmix_all[:, qt, j * 512:(j + 1) * 512], pqj[:, qt, :]
                        )

                for qt in range(NQT):
                    statsq = lnp.tile([P, 3, 6], F32, tag="statsq")
                    for j in range(3):
                        nc.vector.bn_stats(
                            statsq[:, j, :], qmix_all[:, qt, j * 512:(j + 1) * 512]
                        )
                    mvq = lnp.tile([P, 2], F32, tag="mv")
                    nc.vector.bn_aggr(mvq[:], statsq[:])
                    rstdq = lnp.tile([P, 1], F32, tag="rstd")
                    nc.scalar.activation(
                        rstdq[:], mvq[:, 1:2], AF.Sqrt, bias=eps_t[:]
                    )
                    nc.vector.reciprocal(rstdq[:], rstdq[:])
                    nc.vector.tensor_scalar(
                        qln_all[:, qt, :], qmix_all[:, qt, :], mvq[:, 0:1],
                        rstdq[:], op0=OP.subtract, op1=OP.mult,
                    )

            # hidden states + kv weights are dead now; free before phase 1.5
            p1ab.close()

            # ============ segment C: batched q latent transposes ============
            with ExitStack() as pC:
                psC2 = pC.enter_context(tc.tile_pool(name="psC2", bufs=1, space="PSUM"))
                ev = 0
                for qt in range(NQT):
                    for half in range(2):
                        ptq = psC2.tile([P, 6, P], BF16, tag="ptr", bufs=2)
                        for i in range(6):
                            lt = half * 6 + i
                            nc.tensor.transpose(
                                ptq[:, i, :],
                                qln_all[:, qt, lt * P:(lt + 1) * P], ident[:],
                            )
                        dst = qlat_t[:, half * 6:(half + 1) * 6,
                                     qt * P:(qt + 1) * P]
                        if ev % 2 == 0:
                            nc.vector.tensor_copy(dst, ptq[:])
                        else:
                            nc.scalar.copy(dst, ptq[:])
                        ev += 1

            # ===== phase 1.5: q_rope (4 groups) + q_nope (16 heads) staged =====
            with ExitStack() as p15:
                wqrp = p15.enter_context(tc.tile_pool(name="wqrp", bufs=1))
                wqs = p15.enter_context(tc.tile_pool(name="wqs", bufs=3))
                qwork = p15.enter_context(tc.tile_pool(name="qwork", bufs=2))
                ps15 = p15.enter_context(tc.tile_pool(name="ps15", bufs=1, space="PSUM"))

                # preload all q_rope weights, THEN the gathered kfull (which
                # waits on AG1 and would head-of-line block later DMAs)
                wqr_all = wqrp.tile([P, 8, NLT, P], BF16)
                for g in range(G):
                    for half in range(2):
                        col0 = half * 512 + g * P
                        nc.sync.dma_start(
                            wqr_all[:, half * 4 + g, :, :],
                            wqr_t[:, col0:col0 + P].rearrange(
                                "(t p) c -> p t c", p=P
                            ),
                        )
                for a in range(5):
                    nc.sync.dma_start(
                        kfull[:, a, :],
                        kv_out.ap().rearrange(
                            "(r a p) c -> p a r c", r=G, a=5
                        )[:, a],
                    )

                for g in range(G):
                    qraw = qwork.tile([P, 2, CH], F32, tag="qraw")
                    for half in range(2):
                        pr = ps15.tile([P, 512], F32, tag="proj", bufs=2)
                        for lt in range(NLT):
                            nc.tensor.matmul(
                                pr[:], wqr_all[:, half * 4 + g, lt, :],
                                qlat_t[:, lt, :],
                                start=(lt == 0), stop=(lt == NLT - 1),
                            )
                        nc.scalar.add(
                            qraw[:, half, :], pr[:],
                            bqr_t[:, half * 4 + g:half * 4 + g + 1],
                        )
                    tm = qwork.tile([P, CH], F32, tag="tm")
                    tn = qwork.tile([P, CH], F32, tag="tn")
                    x1, x2 = qraw[:, 0, :], qraw[:, 1, :]
                    nc.vector.tensor_tensor(tm[:], x2, sq_t[:], OP.mult)
                    nc.vector.tensor_tensor(tn[:], x1, cq_t[:], OP.mult)
                    nc.vector.tensor_tensor(qro_all[:, g, 0, :], tn[:], tm[:], OP.subtract)
                    nc.vector.tensor_tensor(tm[:], x2, cq_t[:], OP.mult)
                    nc.vector.tensor_tensor(tn[:], x1, sq_t[:], OP.mult)
                    nc.vector.tensor_tensor(qro_all[:, g, 1, :], tn[:], tm[:], OP.add)

                for h in range(H):
                    wb = wqs.tile([P, NLT, P], BF16, tag="wq")
                    nc.sync.dma_start(
                        wb[:],
                        wqb_t[:, h * P:(h + 1) * P].rearrange(
                            "(t p) c -> p t c", p=P
                        ),
                    )
                    pn = ps15.tile([P, 512], F32, tag="proj", bufs=2)
                    for lt in range(NLT):
                        nc.tensor.matmul(
                            pn[:], wb[:, lt, :], qlat_t[:, lt, :],
                            start=(lt == 0), stop=(lt == NLT - 1),
                        )
                    nc.scalar.add(qnope_all[:, h, :], pn[:], bqn_t[:, h:h + 1])

        # ====================== phase 2: attention head loop ======================
        attp = octx.enter_context(tc.tile_pool(name="attp", bufs=1))
        avT = attp.tile([P, H, CH], BF16)

        wo_pre = []
        with ExitStack() as p2:
            hwork = p2.enter_context(tc.tile_pool(name="hwork", bufs=1))
            gwork = p2.enter_context(tc.tile_pool(name="gwork", bufs=2))
            probs_p = p2.enter_context(tc.tile_pool(name="probs_p", bufs=2))
            foldp = p2.enter_context(tc.tile_pool(name="foldp", bufs=3))
            ps2 = p2.enter_context(tc.tile_pool(name="ps2", bufs=1, space="PSUM"))

            def load_knope(h):
                t = hwork.tile([P, S], BF16, tag="knopeT", bufs=2)
                nc.sync.dma_start(
                    t[:],
                    kn_out.ap().rearrange(
                        "(r h p) c -> p h r c", r=G, h=NHG
                    )[:, h - H_GATH],
                )
                return t

            def load_vlat(g):
                t = gwork.tile([P, NKT, CH], BF16, tag="vlatq", bufs=2)
                nc.sync.dma_start(
                    t[:],
                    vl_out.ap().rearrange(
                        "(r t p) (g c) -> p g r t c", r=G, t=NKTC, g=NGG
                    )[:, g - G_GATH],
                )
                return t

            kn_pending = None
            vl_pending = None
            vlatq = None
            for h in range(H):
                g, m = divmod(h, 4)

                if m == 0:
                    if g < G_GATH:
                        # local v_lat for this group from gathered kfull
                        vlatq = gwork.tile([P, NKT, CH], BF16, tag="vlatq",
                                           bufs=2)
                        for kt in range(NKT):
                            pv1 = ps2.tile([P, 512], F32, tag="proj", bufs=2)
                            for rc in range(4):
                                nc.tensor.matmul(
                                    pv1[:], kfull[:, rc, kt * P:(kt + 1) * P],
                                    vupT[:, rc, g * 512:(g + 1) * 512],
                                    start=(rc == 0), stop=(rc == 3),
                                )
                            nc.vector.tensor_tensor(
                                vlatq[:, kt, :], pv1[:],
                                bvv_bc[:, g * 512:(g + 1) * 512], OP.add,
                            )
                    else:
                        vlatq = vl_pending

                if h == H - 1:
                    for i in range(3):
                        wo = wop.tile([P, 512], BF16, tag="wo")
                        nc.sync.dma_start(wo[:], wo_t[i * P:(i + 1) * P, 0:512])
                        wo_pre.append(wo)

                # k_nope^T for this head: local compute or gathered load
                if h < H_GATH:
                    knopeT = hwork.tile([P, S], BF16, tag="knopeT", bufs=2)
                    for kc in range(4):
                        pk = ps2.tile([P, 512], F32, tag="proj", bufs=2)
                        for rc in range(4):
                            nc.tensor.matmul(
                                pk[:], kupT[:, rc, h * P:(h + 1) * P],
                                kfull[:, rc, kc * 512:(kc + 1) * 512],
                                start=(rc == 0), stop=(rc == 3),
                            )
                        nc.scalar.add(
                            knopeT[:, kc * 512:(kc + 1) * 512], pk[:],
                            bkn_t[:, h:h + 1],
                        )
                else:
                    knopeT = kn_pending
                if h + 1 >= H_GATH and h + 1 < H:
                    kn_pending = load_knope(h + 1)

                qnope = qnope_all[:, h, :]
                qro = qro_all[:, g, :, :]
                qropeT = hwork.tile([P, CH], BF16, tag="qropeT", bufs=2)
                nc.sync.dma_start(qropeT[0:32, :], qro[m * 32:(m + 1) * 32, 0, :])
                nc.sync.dma_start(qropeT[32:64, :], qro[m * 32:(m + 1) * 32, 1, :])
                nc.sync.dma_start(qropeT[64:96, :], qro[m * 32:(m + 1) * 32, 0, :])
                nc.sync.dma_start(qropeT[96:128, :], qro[m * 32:(m + 1) * 32, 1, :])

                # prefetch next group's gathered v_lat late (after AG3 lands)
                # so the waiting DMA doesn't head-of-line-block this queue
                if m == 2 and g + 1 >= G_GATH and g + 1 < G:
                    vl_pending = load_vlat(g + 1)

                probs = probs_p.tile([P, NKT, CH], BF16, tag="probs")
                folds = []
                quads = []
                octs = []
                pv = ps2.tile([P, 512], F32, tag="attn", bufs=1)
                pd = ps2.tile([P, 512], F32, tag="den", bufs=1)
                for p in range(NKT // 2):
                    kt, kt1 = 2 * p, 2 * p + 1
                    sc = ps2.tile([P, 2, 512], F32, tag="scores", bufs=2)
                    nc.tensor.matmul(
                        sc[:, 0, :], knopeT[:, kt * P:(kt + 1) * P], qnope,
                        start=True, stop=False,
                    )
                    nc.tensor.matmul(
                        sc[:, 1, :], knopeT[:, kt1 * P:(kt1 + 1) * P], qnope,
                        start=True, stop=False,
                    )
                    nc.tensor.matmul(
                        sc[:, 0, :], kfull[0:DR, 4, kt * P:(kt + 1) * P],
                        qropeT[0:DR, :], start=False, stop=True,
                    )
                    nc.tensor.matmul(
                        sc[:, 1, :], kfull[DR:P, 4, kt1 * P:(kt1 + 1) * P],
                        qropeT[DR:P, :], start=False, stop=True,
                        tile_position=(DR, 0),
                    )
                    nc.scalar.activation(probs[:, kt:kt + 2, :], sc[:], AF.Exp)
                    ft = foldp.tile([P, CH], F32R, tag="fold")
                    nc.vector.tensor_tensor(
                        ft[:], probs[:, kt, :], probs[:, kt1, :], OP.add
                    )
                    folds.append(ft)
                    if p % 2 == 1:
                        fq = foldp.tile([P, CH], F32R, tag="foldq")
                        nc.vector.tensor_tensor(
                            fq[:], folds[p - 1][:], folds[p][:], OP.add
                        )
                        quads.append(fq)
                    if p % 4 == 3:
                        fo = foldp.tile([P, CH], F32R, tag="foldo")
                        nc.vector.tensor_tensor(
                            fo[:], quads[-2][:], quads[-1][:], OP.add
                        )
                        octs.append(fo)
                    if p >= 1:
                        nc.tensor.matmul(
                            pv[:], vlatq[:, kt - 2, m * P:(m + 1) * P],
                            probs[:, kt - 2, :], start=(p == 1), stop=False,
                        )
                        nc.tensor.matmul(
                            pv[:], vlatq[:, kt - 1, m * P:(m + 1) * P],
                            probs[:, kt - 1, :], start=False, stop=False,
                        )
                    if p == 5:
                        nc.tensor.matmul(
                            pd[:], ones_t[:], octs[0][:],
                            start=True, stop=False,
                        )
                nc.tensor.matmul(
                    pv[:], vlatq[:, NKT - 2, m * P:(m + 1) * P],
                    probs[:, NKT - 2, :], start=False, stop=False,
                )
                nc.tensor.matmul(
                    pv[:], vlatq[:, NKT - 1, m * P:(m + 1) * P],
                    probs[:, NKT - 1, :], start=False, stop=True,
                )
                nc.tensor.matmul(
                    pd[:], ones_t[:], octs[1][:], start=False, stop=True,
                )
                recip = hwork.tile([P, CH], F32, tag="recip", bufs=2)
                nc.vector.reciprocal_approx_fast(recip[:], pd[:])
                nc.vector.tensor_tensor(avT[:, h, :], pv[:], recip[:], OP.mult)

        # ================== phase 3: o_proj in quarter passes ==================
        with ExitStack() as p3:
            outp = p3.enter_context(tc.tile_pool(name="outp", bufs=4))
            ps3 = p3.enter_context(tc.tile_pool(name="ps3", bufs=1, space="PSUM"))

            pre = wo_pre
            for quarter in range(4):
                po = ps3.tile([P, NQT, 512], F32, tag="po", bufs=2)
                for kt in range(H):
                    if kt < len(pre):
                        wo = pre[kt]
                    else:
                        wo = wop.tile([P, 512], BF16, tag="wo")
                        nc.sync.dma_start(
                            wo[:],
                            wo_t[kt * P:(kt + 1) * P,
                                 quarter * 512:(quarter + 1) * 512],
                        )
                    for qc in range(NQT):
                        nc.tensor.matmul(
                            po[:, qc, :],
                            avT[:, kt, qc * P:(qc + 1) * P],
                            wo[:],
                            start=(kt == 0), stop=(kt == H - 1),
                        )
                pre = []
                if quarter < 3:
                    for i in range(2):
                        wo = wop.tile([P, 512], BF16, tag="wo")
                        nc.sync.dma_start(
                            wo[:],
                            wo_t[i * P:(i + 1) * P,
                                 (quarter + 1) * 512:(quarter + 2) * 512],
                        )
                        pre.append(wo)
                for qc in range(NQT):
                    ot = outp.tile([P, 512], F32, tag="ot")
                    if qc % 2 == 0:
                        nc.vector.tensor_copy(ot[:], po[:, qc, :])
                    else:
                        nc.scalar.copy(ot[:], po[:, qc, :])
                    nc.sync.dma_start(
                        out_c[
                            qc * P:(qc + 1) * P,
                            quarter * 512:(quarter + 1) * 512,
                        ],
                        ot[:],
                    )

    nc.compile()
    return nc


_NC_CACHE = None


def _get_nc():
    global _NC_CACHE
    if _NC_CACHE is None:
        _NC_CACHE = build_nc()
    return _NC_CACHE


def _prep_in_maps(inputs):
    hidden = np.asarray(inputs["hidden_states"], dtype=np.float32)
    w_qa = np.asarray(inputs["w_qa"], dtype=np.float32)
    ln_qa_g = np.asarray(inputs["ln_qa_g"], dtype=np.float32)
    ln_qa_b = np.asarray(inputs["ln_qa_b"], dtype=np.float32)
    w_qb = np.asarray(inputs["w_qb"], dtype=np.float32)
    w_qrope = np.asarray(inputs["w_qrope"], dtype=np.float32)
    w_kva = np.asarray(inputs["w_kva"], dtype=np.float32)
    ln_kva_g = np.asarray(inputs["ln_kva_g"], dtype=np.float32)
    ln_kva_b = np.asarray(inputs["ln_kva_b"], dtype=np.float32)
    w_kvb = np.asarray(inputs["w_kvb"], dtype=np.float32)
    w_o = np.asarray(inputs["w_o"], dtype=np.float32)
    pos = np.asarray(inputs["position_ids"]).astype(np.int64)

    bf = bfloat16
    hidden_b = hidden.astype(bf)
    hst_all = [
        hidden_b[b].T.reshape(NDT, P, NKT // 2, 2, P).transpose(2, 0, 3, 1, 4)
        for b in range(B)
    ]
    wqa_t = np.ascontiguousarray(w_qa.T.astype(bf))
    # LN gamma folded into q up-projections; beta becomes an output bias:
    # q_nope = (ln0*g + b) @ w_qb.T = ln0 @ (w_qb*g).T + w_qb @ b
    wqb_g = w_qb * ln_qa_g[None, :]
    bqn = (w_qb @ ln_qa_b).astype(np.float32)
    wqb_t = np.ascontiguousarray(wqb_g.T.astype(bf))
    wqr_s = SCALE * w_qrope
    bqr_full = (wqr_s @ ln_qa_b).astype(np.float32)
    wqr_g = (wqr_s * ln_qa_g[None, :]).T
    wqr_t = np.ascontiguousarray(
        wqr_g.reshape(LAT, H, 2, DR // 2).transpose(0, 2, 1, 3)
        .reshape(LAT, H * DR).astype(bf)
    )
    bqr_perm = np.ascontiguousarray(
        bqr_full.reshape(H, 2, DR // 2).transpose(1, 0, 2).reshape(H * DR)
    )
    wkva_t = np.ascontiguousarray(w_kva.T.astype(bf))
    kup = (SCALE * w_kvb[: H * DN]).reshape(H, DN, R)
    bkn = (kup @ ln_kva_b).reshape(H * DN).astype(np.float32)
    kup_g = kup * ln_kva_g[None, None, :]
    kup_t = np.ascontiguousarray(
        kup_g.transpose(2, 0, 1).reshape(R, H * DN).astype(bf)
    )
    vup = w_kvb[H * DN:].reshape(H, DV, R)
    bvv = (vup @ ln_kva_b).reshape(H * DV).astype(np.float32)
    vup_g = vup * ln_kva_g[None, None, :]
    vup_t = np.ascontiguousarray(
        vup_g.transpose(2, 0, 1).reshape(R, H * DV).astype(bf)
    )
    wo_t = np.ascontiguousarray(w_o.T.astype(bf))
    ones_in = np.ones((P, P), dtype=np.float32)

    inv_freq = 1.0 / (10000.0 ** (np.arange(0, DR, 2, dtype=np.float64) / DR))
    ang = pos[:, None].astype(np.float64) * inv_freq[None, :]
    cosf = np.ascontiguousarray(np.cos(ang).astype(np.float32))
    sinf = np.ascontiguousarray(np.sin(ang).astype(np.float32))

    in_maps = []
    for c in range(N_CORES):
        b, ch = divmod(c, NQT)
        qs = ch * CH
        cq = np.ascontiguousarray(np.tile(cosf[qs:qs + CH, :].T, (NQT, 1)))
        sq = np.ascontiguousarray(np.tile(sinf[qs:qs + CH, :].T, (NQT, 1)))
        myp = [2 * ch, 2 * ch + 1]
        in_maps.append({
            "hsqt": np.ascontiguousarray(hst_all[b][myp]),
            "wqa_t": wqa_t,
            "wqb_t": wqb_t,
            "wqr_t": wqr_t,
            "wkva_t": wkva_t,
            "kup_t": kup_t,
            "vup_t": vup_t,
            "wo_t": wo_t,
            "bqn_v": bqn,
            "bqr_v": bqr_perm,
            "bkn_v": bkn,
            "bvv_v": bvv,
            "ones_in": ones_in,
            "ck_tab": np.ascontiguousarray(cosf[qs:qs + CH]),
            "sk_tab": np.ascontiguousarray(sinf[qs:qs + CH]),
            "cq_tab": cq,
            "sq_tab": sq,
        })
    return in_maps


def kernel(**inputs) -> np.ndarray:
    nc = _get_nc()
    in_maps = _prep_in_maps(inputs)
    res = run_bass_kernel_spmd(nc, in_maps, core_ids=list(range(N_CORES)))
    out = np.empty((B, S, D), dtype=np.float32)
    for c in range(N_CORES):
        b, ch = divmod(c, NQT)
        out[b, ch * CH:(ch + 1) * CH, :] = res.results[c]["out_c"]
    return out


# revision 9
# speedup vs baseline: 1.0573x; 1.0066x over previous
"""MLA (multi-head latent attention) prefill kernel for 8 Trainium2 NeuronCores.

Sharding: data-parallel over (batch, query-chunk) for the q path, attention
and o_proj. The KV path is split between recompute and collectives, sized to
the measured ~55 GB/s 4-rank AllGather rate (replica groups [[0,1,2,3],
[4,5,6,7]], one per batch):
 - kv-mix runs only over the core's own 512 keys (the same hidden rows as its
   q chunk, so no cross-chunk hidden-state DMAs at all); the resulting
   latent+rope planes (0.65MB) are AllGathered into the full kfull (AG1).
 - k_nope for heads 0-7 and v_lat for head-groups 0-1 are recomputed per-core
   from kfull inside the head loop (baseline style).
 - k_nope for heads 8-15 and v_lat for groups 2-3 are computed once per core
   over its own 512 keys, AllGathered (AG2/AG3, 4MB each) while the PE chews
   through the q path, and DMA-loaded just-in-time in the head loop.
Collectives run on TOPSP/SDMA silicon and overlap with PE compute; AG2/AG3
have >100us of slack before their consumers. Per-core matmul work drops from
~37.6 GF to ~30 GF with zero PE stalls by construction.

Structure:
 - Segment B': kv-mix over own keys, LN + RoPE, transposes -> kfl (5 planes),
   evict + AG1; local k_nope[h8..15] / v_lat[g2..3] shares -> AG2 / AG3.
 - Segment A: q-mix matmuls + LayerNorm (unchanged from baseline).
 - Segment C: q latent transposes (batched, 6/bank).
 - Phase 1.5: q_rope for all 4 head groups + q_nope for all 16 heads staged
   in SBUF; the gathered kfull is DMA-loaded here (after AG1).
 - Phase 2 head loop: per head, scores contract 128+64 channels; softmax
   denominator via pair->quad->oct folds + two ones-matmuls; attn @ v_lat.
 - Phase 3: o_proj in 4 quarter passes with double-buffered PSUM.
"""

import math
from contextlib import ExitStack

import numpy as np
from ml_dtypes import bfloat16

import concourse.bass as bass
import concourse.tile as tile
from concourse import bacc, mybir
from concourse.bass_utils import run_bass_kernel_spmd
from concourse.masks import make_identity

F32 = mybir.dt.float32
F32R = mybir.dt.float32r
BF16 = mybir.dt.bfloat16
AF = mybir.ActivationFunctionType
OP = mybir.AluOpType

B, S, D = 2, 2048, 2048
H = 16
LAT = 1536
R = 512
DN, DR, DV = 128, 64, 128
EPS = 1e-5
SCALE = 1.0 / math.sqrt(DN + DR)

P = 128
CH = 512
NQT = CH // P      # 4 q tiles per chunk
NKT = S // P       # 16 key tiles total
NKTC = CH // P     # 4 key tiles in own chunk
NDT = D // P
NLT = LAT // P

N_CORES = 8
G = 4              # ranks per replica group (one batch)
RG = [[0, 1, 2, 3], [4, 5, 6, 7]]
H_GATH = 8         # heads [H_GATH..H) use gathered k_nope
G_GATH = 2         # groups [G_GATH..4) use gathered v_lat
NHG = H - H_GATH
NGG = 4 - G_GATH


def _bcast_rows(t, n, length):
    return bass.AP(tensor=t, offset=0, ap=[[0, n], [1, length]])


def build_nc():
    nc = bacc.Bacc(None, target_bir_lowering=False, num_devices=N_CORES)

    hsqt = nc.dram_tensor("hsqt", [2, NDT, 2, P, P], BF16, kind="ExternalInput")
    wqa_t = nc.dram_tensor("wqa_t", [D, LAT], BF16, kind="ExternalInput")
    wqb_t = nc.dram_tensor("wqb_t", [LAT, H * DN], BF16, kind="ExternalInput")
    wqr_t = nc.dram_tensor("wqr_t", [LAT, H * DR], BF16, kind="ExternalInput")
    wkva_t = nc.dram_tensor("wkva_t", [D, R + DR], BF16, kind="ExternalInput")
    kup_t = nc.dram_tensor("kup_t", [R, H * DN], BF16, kind="ExternalInput")
    vup_t = nc.dram_tensor("vup_t", [R, H * DV], BF16, kind="ExternalInput")
    wo_t = nc.dram_tensor("wo_t", [H * DV, D], BF16, kind="ExternalInput")
    bqn_v = nc.dram_tensor("bqn_v", [H * DN], F32, kind="ExternalInput")
    bqr_v = nc.dram_tensor("bqr_v", [H * DR], F32, kind="ExternalInput")
    bkn_v = nc.dram_tensor("bkn_v", [H * DN], F32, kind="ExternalInput")
    bvv_v = nc.dram_tensor("bvv_v", [H * DV], F32, kind="ExternalInput")
    ones_in = nc.dram_tensor("ones_in", [P, P], F32R, kind="ExternalInput")
    ck_tab = nc.dram_tensor("ck_tab", [CH, DR // 2], F32, kind="ExternalInput")
    sk_tab = nc.dram_tensor("sk_tab", [CH, DR // 2], F32, kind="ExternalInput")
    cq_tab = nc.dram_tensor("cq_tab", [P, CH], F32, kind="ExternalInput")
    sq_tab = nc.dram_tensor("sq_tab", [P, CH], F32, kind="ExternalInput")
    out_c = nc.dram_tensor("out_c", [CH, D], F32, kind="ExternalOutput")

    # collective bounce buffers (internal DRAM)
    kv_in = nc.dram_tensor("kv_in", [5 * P, CH], BF16)
    kv_out = nc.dram_tensor("kv_out", [G * 5 * P, CH], BF16)
    kn_in = nc.dram_tensor("kn_in", [NHG * P, CH], BF16)
    kn_out = nc.dram_tensor("kn_out", [G * NHG * P, CH], BF16)
    vl_in = nc.dram_tensor("vl_in", [CH, NGG * CH], BF16)
    vl_out = nc.dram_tensor("vl_out", [G * CH, NGG * CH], BF16)

    with tile.TileContext(nc) as tc, ExitStack() as octx:
        res = octx.enter_context(tc.tile_pool(name="res", bufs=1))
        kfull = res.tile([P, 5, S], BF16)   # gathered latent^T (4) + rope^T (1)
        kupT = res.tile([P, 4, H * DN], BF16)
        vupT = res.tile([P, 4, H * DV], BF16)

        consts = octx.enter_context(tc.tile_pool(name="consts", bufs=1))
        ident = consts.tile([P, P], BF16)
        make_identity(nc, ident)
        ones_t = consts.tile([P, P], F32R)
        eps_t = consts.tile([P, 1], F32)
        nc.vector.memset(eps_t, EPS)
        cq_t = consts.tile([P, CH], F32)
        sq_t = consts.tile([P, CH], F32)
        bqn_t = consts.tile([P, H], F32)
        bqr_t = consts.tile([P, 8], F32)
        bkn_t = consts.tile([P, H], F32)
        bvv_bc = consts.tile([P, H * DV], F32)

        qstage = octx.enter_context(tc.tile_pool(name="qstage", bufs=1))
        qro_all = qstage.tile([P, G, 2, CH], BF16)
        qnope_all = qstage.tile([P, H, CH], BF16)

        wop = octx.enter_context(tc.tile_pool(name="wop", bufs=6))

        with ExitStack() as p1all:
            lnsp = p1all.enter_context(tc.tile_pool(name="lnsp", bufs=1))
            qproj = p1all.enter_context(tc.tile_pool(name="qproj", bufs=1))
            qlat_t = qproj.tile([P, NLT, CH], BF16)

            qln_all = lnsp.tile([P, NQT, LAT], BF16)
            lnf_loc = lnsp.tile([P, NKTC, R], BF16)
            kro_loc = lnsp.tile([P, NKTC, P], BF16)

            p1ab = p1all.enter_context(ExitStack())
            hsqp = p1ab.enter_context(tc.tile_pool(name="hsqp", bufs=1))
            wkvp = p1ab.enter_context(tc.tile_pool(name="wkvp", bufs=1))
            hsq_all = hsqp.tile([P, 2, NDT, 2, P], BF16)
            wkv_all = wkvp.tile([P, NDT, R + DR], BF16)
            ck_t = wkvp.tile([P, NKTC, DR // 2], F32)
            sk_t = wkvp.tile([P, NKTC, DR // 2], F32)

            # =============== segment B': kv-mix on own 512 keys ===============
            with ExitStack() as pB:
                mixp = pB.enter_context(tc.tile_pool(name="mixp", bufs=3))
                lnp = pB.enter_context(tc.tile_pool(name="lnp", bufs=2))
                psB = pB.enter_context(tc.tile_pool(name="psB", bufs=1, space="PSUM"))

                # interleave kv weights with own hidden-state tiles so the
                # first matmul can start after ~1.1MB of DMA
                for a in range(4):
                    nc.sync.dma_start(
                        wkv_all[:, 4 * a:4 * (a + 1), :],
                        wkva_t[512 * a:512 * (a + 1), :].rearrange(
                            "(t p) c -> p t c", p=P
                        ),
                    )
                    for pair in range(2):
                        nc.sync.dma_start(
                            hsq_all[:, pair, 4 * a:4 * (a + 1), :, :],
                            hsqt[pair, 4 * a:4 * (a + 1)].rearrange(
                                "d k p c -> p d k c"
                            ),
                        )
                nc.sync.dma_start(
                    ck_t[:], ck_tab.ap().rearrange("(t p) j -> p t j", p=P)
                )
                nc.sync.dma_start(
                    sk_t[:], sk_tab.ap().rearrange("(t p) j -> p t j", p=P)
                )

                # phase-2 weights streamed during kv-mix
                def _stream_weights(dt):
                    if dt % 4 == 1:
                        rc = dt // 4
                        nc.sync.dma_start(
                            kupT[:, rc, :], kup_t[rc * P:(rc + 1) * P, :]
                        )
                    elif dt % 4 == 3:
                        rc = dt // 4
                        nc.sync.dma_start(
                            vupT[:, rc, :], vup_t[rc * P:(rc + 1) * P, :]
                        )
                    elif dt == 2:
                        nc.sync.dma_start(ones_t[:], ones_in[:, :])
                        nc.sync.dma_start(
                            bkn_t[:], bkn_v.ap().rearrange("(h p) -> p h", p=P)
                        )
                        nc.sync.dma_start(bvv_bc[:], _bcast_rows(bvv_v, P, H * DV))
                    elif dt == 6:
                        nc.sync.dma_start(
                            bqn_t[:], bqn_v.ap().rearrange("(h p) -> p h", p=P)
                        )
                        nc.sync.dma_start(
                            bqr_t[:], bqr_v.ap().rearrange("(a p) -> p a", p=P)
                        )
                        nc.sync.dma_start(cq_t[:], cq_tab[:, :])
                        nc.sync.dma_start(sq_t[:], sq_tab[:, :])

                for ktp in range(2):
                    pm = [
                        psB.tile([P, 2, 512], F32, tag=f"pmix{i}", bufs=2,
                                 name=f"pm{i}")
                        for i in range(2)
                    ]
                    for a in range(4):
                        hk8 = hsq_all[:, ktp, 4 * a:4 * (a + 1), :, :]
                        for di in range(4):
                            dt = 4 * a + di
                            st = (dt == 0)
                            sp = (dt == NDT - 1)
                            for i in range(2):
                                nc.tensor.matmul(
                                    pm[i][:, 0, 0:288], hk8[:, di, i, :],
                                    wkv_all[:, dt, 0:288], start=st, stop=sp,
                                )
                                nc.tensor.matmul(
                                    pm[i][:, 1, 0:288], hk8[:, di, i, :],
                                    wkv_all[:, dt, 288:576], start=st, stop=sp,
                                )
                            if ktp == 0:
                                _stream_weights(dt)
                    for i in range(2):
                        kt = 2 * ktp + i
                        kvmix = mixp.tile([P, R + DR], F32, tag="kvmix")
                        nc.scalar.copy(kvmix[:, 0:288], pm[i][:, 0, 0:288])
                        nc.scalar.copy(kvmix[:, 288:576], pm[i][:, 1, 0:288])

                        stats = lnp.tile([P, 6], F32, tag="stats")
                        nc.vector.bn_stats(stats[:], kvmix[:, 0:R])
                        mv = lnp.tile([P, 2], F32, tag="mv")
                        nc.vector.bn_aggr(mv[:], stats[:])
                        rstd = lnp.tile([P, 1], F32, tag="rstd")
                        nc.scalar.activation(
                            rstd[:], mv[:, 1:2], AF.Sqrt, bias=eps_t[:]
                        )
                        nc.vector.reciprocal(rstd[:], rstd[:])
                        nc.vector.tensor_scalar(
                            lnf_loc[:, kt, :], kvmix[:, 0:R], mv[:, 0:1],
                            rstd[:], op0=OP.subtract, op1=OP.mult,
                        )

                        # RoPE, rotated pairs duplicated to cols 64:128
                        t1 = lnp.tile([P, DR // 2], F32, tag="t1")
                        t2 = lnp.tile([P, DR // 2], F32, tag="t2")
                        x1 = kvmix[:, R:R + 32]
                        x2 = kvmix[:, R + 32:R + 64]
                        kro = kro_loc[:, kt, :]
                        nc.vector.tensor_tensor(t2[:], x1, ck_t[:, kt, :], OP.mult)
                        nc.vector.tensor_tensor(t1[:], x2, sk_t[:, kt, :], OP.mult)
                        nc.vector.tensor_tensor(kro[:, 0:32], t2[:], t1[:], OP.subtract)
                        nc.vector.tensor_tensor(kro[:, 64:96], t2[:], t1[:], OP.subtract)
                        nc.vector.tensor_tensor(t2[:], x1, sk_t[:, kt, :], OP.mult)
                        nc.vector.tensor_tensor(t1[:], x2, ck_t[:, kt, :], OP.mult)
                        nc.vector.tensor_tensor(kro[:, 32:64], t2[:], t1[:], OP.add)
                        nc.vector.tensor_tensor(kro[:, 96:128], t2[:], t1[:], OP.add)

            # == segment B2: transposes + AG1; gathered-share k_nope / v_lat ==
            with ExitStack() as pB2:
                klocp = pB2.enter_context(tc.tile_pool(name="klocp", bufs=1))
                psC = pB2.enter_context(tc.tile_pool(name="psC", bufs=1, space="PSUM"))

                kfl = klocp.tile([P, 5, CH], BF16)
                kng = klocp.tile([P, NHG, CH], BF16)
                vlg = klocp.tile([P, NKTC, NGG * CH], BF16)

                for kt in range(NKTC):
                    pt = psC.tile([P, 5, P], BF16, tag="ptr", bufs=2)
                    for j in range(4):
                        nc.tensor.transpose(
                            pt[:, j, :],
                            lnf_loc[:, kt, j * P:(j + 1) * P], ident[:],
                        )
                    nc.tensor.transpose(pt[:, 4, :], kro_loc[:, kt, :], ident[:])
                    dst = kfl[:, 0:5, kt * P:(kt + 1) * P]
                    if kt % 2 == 0:
                        nc.vector.tensor_copy(dst, pt[:])
                    else:
                        nc.scalar.copy(dst, pt[:])

                nc.scalar.dma_start(
                    kv_in.ap().rearrange("(a p) c -> p a c", p=P), kfl[:]
                )
                nc.gpsimd.collective_compute(
                    "AllGather", OP.bypass, replica_groups=RG,
                    ins=[kv_in.ap().opt()], outs=[kv_out.ap().opt()],
                )

                # gathered-share k_nope: heads 8..15 x own 512 keys
                for j in range(NHG):
                    h = H_GATH + j
                    pk = psC.tile([P, CH], F32, tag="pk", bufs=2)
                    for rc in range(4):
                        nc.tensor.matmul(
                            pk[:], kupT[:, rc, h * P:(h + 1) * P],
                            kfl[:, rc, :],
                            start=(rc == 0), stop=(rc == 3),
                        )
                    nc.scalar.add(kng[:, j, :], pk[:], bkn_t[:, h:h + 1])
                nc.scalar.dma_start(
                    kn_in.ap().rearrange("(h p) c -> p h c", p=P), kng[:]
                )
                nc.gpsimd.collective_compute(
                    "AllGather", OP.bypass, replica_groups=RG,
                    ins=[kn_in.ap().opt()], outs=[kn_out.ap().opt()],
                )

                # gathered-share v_lat: groups 2..3 x own 512 keys
                for kt in range(NKTC):
                    for gg in range(NGG):
                        g = G_GATH + gg
                        pvv = psC.tile([P, CH], F32, tag="pvv", bufs=2)
                        for rc in range(4):
                            nc.tensor.matmul(
                                pvv[:], kfl[:, rc, kt * P:(kt + 1) * P],
                                vupT[:, rc, g * 512:(g + 1) * 512],
                                start=(rc == 0), stop=(rc == 3),
                            )
                        nc.vector.tensor_tensor(
                            vlg[:, kt, gg * 512:(gg + 1) * 512], pvv[:],
                            bvv_bc[:, g * 512:(g + 1) * 512], OP.add,
                        )
                nc.scalar.dma_start(
                    vl_in.ap().rearrange("(t p) c -> p t c", p=P), vlg[:]
                )
                nc.gpsimd.collective_compute(
                    "AllGather", OP.bypass, replica_groups=RG,
                    ins=[vl_in.ap().opt()], outs=[vl_out.ap().opt()],
                )

            # ================= segment A: q-mix matmuls + LN =================
            with ExitStack() as pA:
                wqap = pA.enter_context(tc.tile_pool(name="wqap", bufs=4))
                mixp = pA.enter_context(tc.tile_pool(name="mixp", bufs=1))
                lnp = pA.enter_context(tc.tile_pool(name="lnp", bufs=2))
                psA = pA.enter_context(tc.tile_pool(name="psA", bufs=1, space="PSUM"))

                qmix_all = mixp.tile([P, NQT, LAT], BF16)
                for j in range(3):
                    pqj = psA.tile([P, NQT, 512], F32, tag="pq", bufs=2)
                    for a in range(4):
                        wqa_c = wqap.tile([P, 4, 512], BF16, tag="wqa")
                        nc.sync.dma_start(
                            wqa_c[:],
                            wqa_t[a * 512:(a + 1) * 512,
                                  j * 512:(j + 1) * 512].rearrange(
                                "(t p) c -> p t c", p=P
                            ),
                        )
                        for i in range(4):
                            dt = 4 * a + i
                            for qt in range(NQT):
                                nc.tensor.matmul(
                                    pqj[:, qt, :],
                                    hsq_all[:, qt // 2, dt, qt % 2, :],
                                    wqa_c[:, i, :],
                                    start=(dt == 0), stop=(dt == NDT - 1),
                                )
                    for qt in range(NQT):
                        nc.vector.tensor_copy(
                            qmix_all[:, qt, j * 512:(j + 1) * 512], pqj[:, qt, :]
                        )

                for qt in range(NQT):
                    statsq = lnp.tile([P, 3, 6], F32, tag="statsq")
                    for j in range(3):
                        nc.vector.bn_stats(
                            statsq[:, j, :], qmix_all[:, qt, j * 512:(j + 1) * 512]
                        )
                    mvq = lnp.tile([P, 2], F32, tag="mv")
                    nc.vector.bn_aggr(mvq[:], statsq[:])
                    rstdq = lnp.tile([P, 1], F32, tag="rstd")
                    nc.scalar.activation(
                        rstdq[:], mvq[:, 1:2], AF.Sqrt, bias=eps_t[:]
                    )
                    nc.vector.reciprocal(rstdq[:], rstdq[:])
                    nc.vector.tensor_scalar(
                        qln_all[:, qt, :], qmix_all[:, qt, :], mvq[:, 0:1],
                        rstdq[:], op0=OP.subtract, op1=OP.mult,
                    )

            # hidden states + kv weights are dead now; free before phase 1.5
            p1ab.close()

            # ============ segment C: batched q latent transposes ============
            with ExitStack() as pC:
                psC2 = pC.enter_context(tc.tile_pool(name="psC2", bufs=1, space="PSUM"))
                ev = 0
                for qt in range(NQT):
                    for half in range(2):
                        ptq = psC2.tile([P, 6, P], BF16, tag="ptr", bufs=2)
                        for i in range(6):
                            lt = half * 6 + i
                            nc.tensor.transpose(
                                ptq[:, i, :],
                                qln_all[:, qt, lt * P:(lt + 1) * P], ident[:],
                            )
                        dst = qlat_t[:, half * 6:(half + 1) * 6,
                                     qt * P:(qt + 1) * P]
                        if ev % 2 == 0:
                            nc.vector.tensor_copy(dst, ptq[:])
                        else:
                            nc.scalar.copy(dst, ptq[:])
                        ev += 1

            # ===== phase 1.5: q_rope (4 groups) + q_nope (16 heads) staged =====
            with ExitStack() as p15:
                wqrp = p15.enter_context(tc.tile_pool(name="wqrp", bufs=1))
                wqs = p15.enter_context(tc.tile_pool(name="wqs", bufs=3))
                qwork = p15.enter_context(tc.tile_pool(name="qwork", bufs=2))
                ps15 = p15.enter_context(tc.tile_pool(name="ps15", bufs=1, space="PSUM"))

                # preload all q_rope weights, THEN the gathered kfull (which
                # waits on AG1 and would head-of-line block later DMAs)
                wqr_all = wqrp.tile([P, 8, NLT, P], BF16)
                for g in range(G):
                    for half in range(2):
                        col0 = half * 512 + g * P
                        nc.sync.dma_start(
                            wqr_all[:, half * 4 + g, :, :],
                            wqr_t[:, col0:col0 + P].rearrange(
                                "(t p) c -> p t c", p=P
                            ),
                        )
                for a in range(5):
                    nc.scalar.dma_start(
                        kfull[:, a, :],
                        kv_out.ap().rearrange(
                            "(r a p) c -> p a r c", r=G, a=5
                        )[:, a],
                    )

                for g in range(G):
                    qraw = qwork.tile([P, 2, CH], F32, tag="qraw")
                    for half in range(2):
                        pr = ps15.tile([P, 512], F32, tag="proj", bufs=2)
                        for lt in range(NLT):
                            nc.tensor.matmul(
                                pr[:], wqr_all[:, half * 4 + g, lt, :],
                                qlat_t[:, lt, :],
                                start=(lt == 0), stop=(lt == NLT - 1),
                            )
                        nc.scalar.add(
                            qraw[:, half, :], pr[:],
                            bqr_t[:, half * 4 + g:half * 4 + g + 1],
                        )
                    tm = qwork.tile([P, CH], F32, tag="tm")
                    tn = qwork.tile([P, CH], F32, tag="tn")
                    x1, x2 = qraw[:, 0, :], qraw[:, 1, :]
                    nc.vector.tensor_tensor(tm[:], x2, sq_t[:], OP.mult)
                    nc.vector.tensor_tensor(tn[:], x1, cq_t[:], OP.mult)
                    nc.vector.tensor_tensor(qro_all[:, g, 0, :], tn[:], tm[:], OP.subtract)
                    nc.vector.tensor_tensor(tm[:], x2, cq_t[:], OP.mult)
                    nc.vector.tensor_tensor(tn[:], x1, sq_t[:], OP.mult)
                    nc.vector.tensor_tensor(qro_all[:, g, 1, :], tn[:], tm[:], OP.add)

                for h in range(H):
                    wb = wqs.tile([P, NLT, P], BF16, tag="wq")
                    nc.sync.dma_start(
                        wb[:],
                        wqb_t[:, h * P:(h + 1) * P].rearrange(
                            "(t p) c -> p t c", p=P
                        ),
                    )
                    pn = ps15.tile([P, 512], F32, tag="proj", bufs=2)
                    for lt in range(NLT):
                        nc.tensor.matmul(
                            pn[:], wb[:, lt, :], qlat_t[:, lt, :],
                            start=(lt == 0), stop=(lt == NLT - 1),
                        )
                    nc.scalar.add(qnope_all[:, h, :], pn[:], bqn_t[:, h:h + 1])

        # ====================== phase 2: attention head loop ======================
        attp = octx.enter_context(tc.tile_pool(name="attp", bufs=1))
        avT = attp.tile([P, H, CH], BF16)

        wo_pre = []
        with ExitStack() as p2:
            hwork = p2.enter_context(tc.tile_pool(name="hwork", bufs=1))
            gwork = p2.enter_context(tc.tile_pool(name="gwork", bufs=2))
            probs_p = p2.enter_context(tc.tile_pool(name="probs_p", bufs=2))
            foldp = p2.enter_context(tc.tile_pool(name="foldp", bufs=3))
            ps2 = p2.enter_context(tc.tile_pool(name="ps2", bufs=1, space="PSUM"))

            def load_knope(h):
                t = hwork.tile([P, S], BF16, tag="knopeT", bufs=2)
                nc.scalar.dma_start(
                    t[:],
                    kn_out.ap().rearrange(
                        "(r h p) c -> p h r c", r=G, h=NHG
                    )[:, h - H_GATH],
                )
                return t

            def load_vlat(g):
                t = gwork.tile([P, NKT, CH], BF16, tag="vlatq", bufs=2)
                nc.scalar.dma_start(
                    t[:],
                    vl_out.ap().rearrange(
                        "(r t p) (g c) -> p g r t c", r=G, t=NKTC, g=NGG
                    )[:, g - G_GATH],
                )
                return t

            kn_pending = None
            vl_pending = None
            vlatq = None
            for h in range(H):
                g, m = divmod(h, 4)

                if m == 0:
                    if g < G_GATH:
                        # local v_lat for this group from gathered kfull
                        vlatq = gwork.tile([P, NKT, CH], BF16, tag="vlatq",
                                           bufs=2)
                        for kt in range(NKT):
                            pv1 = ps2.tile([P, 512], F32, tag="proj", bufs=2)
                            for rc in range(4):
                                nc.tensor.matmul(
                                    pv1[:], kfull[:, rc, kt * P:(kt + 1) * P],
                                    vupT[:, rc, g * 512:(g + 1) * 512],
                                    start=(rc == 0), stop=(rc == 3),
                                )
                            nc.vector.tensor_tensor(
                                vlatq[:, kt, :], pv1[:],
                                bvv_bc[:, g * 512:(g + 1) * 512], OP.add,
                            )
                    else:
                        vlatq = vl_pending

                if h == H - 1:
                    for i in range(3):
                        wo = wop.tile([P, 512], BF16, tag="wo")
                        nc.sync.dma_start(wo[:], wo_t[i * P:(i + 1) * P, 0:512])
                        wo_pre.append(wo)

                # k_nope^T for this head: local compute or gathered load
                if h < H_GATH:
                    knopeT = hwork.tile([P, S], BF16, tag="knopeT", bufs=2)
                    for kc in range(4):
                        pk = ps2.tile([P, 512], F32, tag="proj", bufs=2)
                        for rc in range(4):
                            nc.tensor.matmul(
                                pk[:], kupT[:, rc, h * P:(h + 1) * P],
                                kfull[:, rc, kc * 512:(kc + 1) * 512],
                                start=(rc == 0), stop=(rc == 3),
                            )
                        nc.scalar.add(
                            knopeT[:, kc * 512:(kc + 1) * 512], pk[:],
                            bkn_t[:, h:h + 1],
                        )
                else:
                    knopeT = kn_pending
                if h + 1 >= H_GATH and h + 1 < H:
                    kn_pending = load_knope(h + 1)

                qnope = qnope_all[:, h, :]
                qro = qro_all[:, g, :, :]
                qropeT = hwork.tile([P, CH], BF16, tag="qropeT", bufs=2)
                nc.sync.dma_start(qropeT[0:32, :], qro[m * 32:(m + 1) * 32, 0, :])
                nc.sync.dma_start(qropeT[32:64, :], qro[m * 32:(m + 1) * 32, 1, :])
                nc.sync.dma_start(qropeT[64:96, :], qro[m * 32:(m + 1) * 32, 0, :])
                nc.sync.dma_start(qropeT[96:128, :], qro[m * 32:(m + 1) * 32, 1, :])

                # prefetch next group's gathered v_lat late (after AG3 lands)
                # so the waiting DMA doesn't head-of-line-block this queue
                if m == 2 and g + 1 >= G_GATH and g + 1 < G:
                    vl_pending = load_vlat(g + 1)

                probs = probs_p.tile([P, NKT, CH], BF16, tag="probs")
                folds = []
                quads = []
                octs = []
                pv = ps2.tile([P, 512], F32, tag="attn", bufs=1)
                pd = ps2.tile([P, 512], F32, tag="den", bufs=1)
                for p in range(NKT // 2):
                    kt, kt1 = 2 * p, 2 * p + 1
                    sc = ps2.tile([P, 2, 512], F32, tag="scores", bufs=2)
                    nc.tensor.matmul(
                        sc[:, 0, :], knopeT[:, kt * P:(kt + 1) * P], qnope,
                        start=True, stop=False,
                    )
                    nc.tensor.matmul(
                        sc[:, 1, :], knopeT[:, kt1 * P:(kt1 + 1) * P], qnope,
                        start=True, stop=False,
                    )
                    nc.tensor.matmul(
                        sc[:, 0, :], kfull[0:DR, 4, kt * P:(kt + 1) * P],
                        qropeT[0:DR, :], start=False, stop=True,
                    )
                    nc.tensor.matmul(
                        sc[:, 1, :], kfull[DR:P, 4, kt1 * P:(kt1 + 1) * P],
                        qropeT[DR:P, :], start=False, stop=True,
                        tile_position=(DR, 0),
                    )
                    nc.scalar.activation(probs[:, kt:kt + 2, :], sc[:], AF.Exp)
                    ft = foldp.tile([P, CH], F32R, tag="fold")
                    nc.vector.tensor_tensor(
                        ft[:], probs[:, kt, :], probs[:, kt1, :], OP.add
                    )
                    folds.append(ft)
                    if p % 2 == 1:
                        fq = foldp.tile([P, CH], F32R, tag="foldq")
                        nc.vector.tensor_tensor(
                            fq[:], folds[p - 1][:], folds[p][:], OP.add
                        )
                        quads.append(fq)
                    if p % 4 == 3:
                        fo = foldp.tile([P, CH], F32R, tag="foldo")
                        nc.vector.tensor_tensor(
                            fo[:], quads[-2][:], quads[-1][:], OP.add
                        )
                        octs.append(fo)
                    if p >= 1:
                        nc.tensor.matmul(
                            pv[:], vlatq[:, kt - 2, m * P:(m + 1) * P],
                            probs[:, kt - 2, :], start=(p == 1), stop=False,
                        )
                        nc.tensor.matmul(
                            pv[:], vlatq[:, kt - 1, m * P:(m + 1) * P],
                            probs[:, kt - 1, :], start=False, stop=False,
                        )
                    if p == 5:
                        nc.tensor.matmul(
                            pd[:], ones_t[:], octs[0][:],
                            start=True, stop=False,
                        )
                nc.tensor.matmul(
                    pv[:], vlatq[:, NKT - 2, m * P:(m + 1) * P],
                    probs[:, NKT - 2, :], start=False, stop=False,
                )
                nc.tensor.matmul(
                    pv[:], vlatq[:, NKT - 1, m * P:(m + 1) * P],
                    probs[:, NKT - 1, :], start=False, stop=True,
                )
                nc.tensor.matmul(
                    pd[:], ones_t[:], octs[1][:], start=False, stop=True,
                )
                recip = hwork.tile([P, CH], F32, tag="recip", bufs=2)
                nc.vector.reciprocal_approx_fast(recip[:], pd[:])
                nc.vector.tensor_tensor(avT[:, h, :], pv[:], recip[:], OP.mult)

        # ================== phase 3: o_proj in quarter passes ==================
        with ExitStack() as p3:
            outp = p3.enter_context(tc.tile_pool(name="outp", bufs=4))
            ps3 = p3.enter_context(tc.tile_pool(name="ps3", bufs=1, space="PSUM"))

            pre = wo_pre
            for quarter in range(4):
                po = ps3.tile([P, NQT, 512], F32, tag="po", bufs=2)
                for kt in range(H):
                    if kt < len(pre):
                        wo = pre[kt]
                    else:
                        wo = wop.tile([P, 512], BF16, tag="wo")
                        nc.sync.dma_start(
                            wo[:],
                            wo_t[kt * P:(kt + 1) * P,
                                 quarter * 512:(quarter + 1) * 512],
                        )
                    for qc in range(NQT):
                        nc.tensor.matmul(
                            po[:, qc, :],
                            avT[:, kt, qc * P:(qc + 1) * P],
                            wo[:],
                            start=(kt == 0), stop=(kt == H - 1),
                        )
                pre = []
                if quarter < 3:
                    for i in range(2):
                        wo = wop.tile([P, 512], BF16, tag="wo")
                        nc.sync.dma_start(
                            wo[:],
                            wo_t[i * P:(i + 1) * P,
                                 (quarter + 1) * 512:(quarter + 2) * 512],
                        )
                        pre.append(wo)
                for qc in range(NQT):
                    ot = outp.tile([P, 512], F32, tag="ot")
                    if qc % 2 == 0:
                        nc.vector.tensor_copy(ot[:], po[:, qc, :])
                    else:
                        nc.scalar.copy(ot[:], po[:, qc, :])
                    nc.sync.dma_start(
                        out_c[
                            qc * P:(qc + 1) * P,
                            quarter * 512:(quarter + 1) * 512,
                        ],
                        ot[:],
                    )

    nc.compile()
    return nc


_NC_CACHE = None


def _get_nc():
    global _NC_CACHE
    if _NC_CACHE is None:
        _NC_CACHE = build_nc()
    return _NC_CACHE


def _prep_in_maps(inputs):
    hidden = np.asarray(inputs["hidden_states"], dtype=np.float32)
    w_qa = np.asarray(inputs["w_qa"], dtype=np.float32)
    ln_qa_g = np.asarray(inputs["ln_qa_g"], dtype=np.float32)
    ln_qa_b = np.asarray(inputs["ln_qa_b"], dtype=np.float32)
    w_qb = np.asarray(inputs["w_qb"], dtype=np.float32)
    w_qrope = np.asarray(inputs["w_qrope"], dtype=np.float32)
    w_kva = np.asarray(inputs["w_kva"], dtype=np.float32)
    ln_kva_g = np.asarray(inputs["ln_kva_g"], dtype=np.float32)
    ln_kva_b = np.asarray(inputs["ln_kva_b"], dtype=np.float32)
    w_kvb = np.asarray(inputs["w_kvb"], dtype=np.float32)
    w_o = np.asarray(inputs["w_o"], dtype=np.float32)
    pos = np.asarray(inputs["position_ids"]).astype(np.int64)

    bf = bfloat16
    hidden_b = hidden.astype(bf)
    hst_all = [
        hidden_b[b].T.reshape(NDT, P, NKT // 2, 2, P).transpose(2, 0, 3, 1, 4)
        for b in range(B)
    ]
    wqa_t = np.ascontiguousarray(w_qa.T.astype(bf))
    # LN gamma folded into q up-projections; beta becomes an output bias:
    # q_nope = (ln0*g + b) @ w_qb.T = ln0 @ (w_qb*g).T + w_qb @ b
    wqb_g = w_qb * ln_qa_g[None, :]
    bqn = (w_qb @ ln_qa_b).astype(np.float32)
    wqb_t = np.ascontiguousarray(wqb_g.T.astype(bf))
    wqr_s = SCALE * w_qrope
    bqr_full = (wqr_s @ ln_qa_b).astype(np.float32)
    wqr_g = (wqr_s * ln_qa_g[None, :]).T
    wqr_t = np.ascontiguousarray(
        wqr_g.reshape(LAT, H, 2, DR // 2).transpose(0, 2, 1, 3)
        .reshape(LAT, H * DR).astype(bf)
    )
    bqr_perm = np.ascontiguousarray(
        bqr_full.reshape(H, 2, DR // 2).transpose(1, 0, 2).reshape(H * DR)
    )
    wkva_t = np.ascontiguousarray(w_kva.T.astype(bf))
    kup = (SCALE * w_kvb[: H * DN]).reshape(H, DN, R)
    bkn = (kup @ ln_kva_b).reshape(H * DN).astype(np.float32)
    kup_g = kup * ln_kva_g[None, None, :]
    kup_t = np.ascontiguousarray(
        kup_g.transpose(2, 0, 1).reshape(R, H * DN).astype(bf)
    )
    vup = w_kvb[H * DN:].reshape(H, DV, R)
    bvv = (vup @ ln_kva_b).reshape(H * DV).astype(np.float32)
    vup_g = vup * ln_kva_g[None, None, :]
    vup_t = np.ascontiguousarray(
        vup_g.transpose(2, 0, 1).reshape(R, H * DV).astype(bf)
    )
    wo_t = np.ascontiguousarray(w_o.T.astype(bf))
    ones_in = np.ones((P, P), dtype=np.float32)

    inv_freq = 1.0 / (10000.0 ** (np.arange(0, DR, 2, dtype=np.float64) / DR))
    ang = pos[:, None].astype(np.float64) * inv_freq[None, :]
    cosf = np.ascontiguousarray(np.cos(ang).astype(np.float32))
    sinf = np.ascontiguousarray(np.sin(ang).astype(np.float32))

    in_maps = []
    for c in range(N_CORES):
        b, ch = divmod(c, NQT)
        qs = ch * CH
        cq = np.ascontiguousarray(np.tile(cosf[qs:qs + CH, :].T, (NQT, 1)))
        sq = np.ascontiguousarray(np.tile(sinf[qs:qs + CH, :].T, (NQT, 1)))
        myp = [2 * ch, 2 * ch + 1]
        in_maps.append({
            "hsqt": np.ascontiguousarray(hst_all[b][myp]),
            "wqa_t": wqa_t,
            "wqb_t": wqb_t,
            "wqr_t": wqr_t,
            "wkva_t": wkva_t,
            "kup_t": kup_t,
            "vup_t": vup_t,
            "wo_t": wo_t,
            "bqn_v": bqn,
            "bqr_v": bqr_perm,
            "bkn_v": bkn,
            "bvv_v": bvv,
            "ones_in": ones_in,
            "ck_tab": np.ascontiguousarray(cosf[qs:qs + CH]),
            "sk_tab": np.ascontiguousarray(sinf[qs:qs + CH]),
            "cq_tab": cq,
            "sq_tab": sq,
        })
    return in_maps


def kernel(**inputs) -> np.ndarray:
    nc = _get_nc()
    in_maps = _prep_in_maps(inputs)
    res = run_bass_kernel_spmd(nc, in_maps, core_ids=list(range(N_CORES)))
    out = np.empty((B, S, D), dtype=np.float32)
    for c in range(N_CORES):
        b, ch = divmod(c, NQT)
        out[b, ch * CH:(ch + 1) * CH, :] = res.results[c]["out_c"]
    return out


# revision 11
# speedup vs baseline: 1.0951x; 1.0357x over previous
"""MLA (multi-head latent attention) prefill kernel for 8 Trainium2 NeuronCores.

Sharding: data-parallel over (batch, query-chunk) for the q path, attention
and o_proj. The KV path is split between recompute and collectives, sized to
the measured ~55 GB/s 4-rank AllGather rate (replica groups [[0,1,2,3],
[4,5,6,7]], one per batch):
 - kv-mix runs only over the core's own 512 keys (the same hidden rows as its
   q chunk, so no cross-chunk hidden-state DMAs at all); the resulting
   latent+rope planes (0.65MB) are AllGathered into the full kfull (AG1).
 - k_nope for heads 0-7 and v_lat for head-groups 0-1 are recomputed per-core
   from kfull inside the head loop (baseline style).
 - k_nope for heads 8-15 and v_lat for groups 2-3 are computed once per core
   over its own 512 keys, AllGathered (AG2/AG3, 4MB each) while the PE chews
   through the q path, and DMA-loaded just-in-time in the head loop.
Collectives run on TOPSP/SDMA silicon and overlap with PE compute; AG2/AG3
have >100us of slack before their consumers. Per-core matmul work drops from
~37.6 GF to ~30 GF with zero PE stalls by construction.

Structure:
 - Segment B': kv-mix over own keys, LN + RoPE, transposes -> kfl (5 planes),
   evict + AG1; local k_nope[h8..15] / v_lat[g2..3] shares -> AG2 / AG3.
 - Segment A: q-mix matmuls + LayerNorm (unchanged from baseline).
 - Segment C: q latent transposes (batched, 6/bank).
 - Phase 1.5: q_rope for all 4 head groups + q_nope for all 16 heads staged
   in SBUF; the gathered kfull is DMA-loaded here (after AG1).
 - Phase 2 head loop: per head, scores contract 128+64 channels; softmax
   denominator via pair->quad->oct folds + two ones-matmuls; attn @ v_lat.
 - Phase 3: o_proj in 4 quarter passes with double-buffered PSUM.
"""

import math
from contextlib import ExitStack

import numpy as np
from ml_dtypes import bfloat16

import concourse.bass as bass
import concourse.tile as tile
from concourse import bacc, mybir
from concourse.bass_utils import run_bass_kernel_spmd
from concourse.masks import make_identity

F32 = mybir.dt.float32
F32R = mybir.dt.float32r
BF16 = mybir.dt.bfloat16
AF = mybir.ActivationFunctionType
OP = mybir.AluOpType

B, S, D = 2, 2048, 2048
H = 16
LAT = 1536
R = 512
DN, DR, DV = 128, 64, 128
EPS = 1e-5
SCALE = 1.0 / math.sqrt(DN + DR)

P = 128
CH = 512
NQT = CH // P      # 4 q tiles per chunk
NKT = S // P       # 16 key tiles total
NKTC = CH // P     # 4 key tiles in own chunk
NDT = D // P
NLT = LAT // P

N_CORES = 8
G = 4              # ranks per replica group (one batch)
RG = [[0, 1, 2, 3], [4, 5, 6, 7]]
H_GATH = 12        # heads [H_GATH..H) use gathered k_nope
G_GATH = 2         # groups [G_GATH..4) use gathered v_lat
NHG = H - H_GATH
NGG = 4 - G_GATH


def _bcast_rows(t, n, length):
    return bass.AP(tensor=t, offset=0, ap=[[0, n], [1, length]])


def build_nc():
    nc = bacc.Bacc(None, target_bir_lowering=False, num_devices=N_CORES)

    hsqt = nc.dram_tensor("hsqt", [2, NDT, 2, P, P], BF16, kind="ExternalInput")
    wqa_t = nc.dram_tensor("wqa_t", [D, LAT], BF16, kind="ExternalInput")
    wqb_t = nc.dram_tensor("wqb_t", [LAT, H * DN], BF16, kind="ExternalInput")
    wqr_t = nc.dram_tensor("wqr_t", [LAT, H * DR], BF16, kind="ExternalInput")
    wkva_t = nc.dram_tensor("wkva_t", [D, R + DR], BF16, kind="ExternalInput")
    kup_t = nc.dram_tensor("kup_t", [R, H * DN], BF16, kind="ExternalInput")
    vup_t = nc.dram_tensor("vup_t", [R, H * DV], BF16, kind="ExternalInput")
    wo_t = nc.dram_tensor("wo_t", [H * DV, D], BF16, kind="ExternalInput")
    bqn_v = nc.dram_tensor("bqn_v", [H * DN], F32, kind="ExternalInput")
    bqr_v = nc.dram_tensor("bqr_v", [H * DR], F32, kind="ExternalInput")
    bkn_v = nc.dram_tensor("bkn_v", [H * DN], F32, kind="ExternalInput")
    bvv_v = nc.dram_tensor("bvv_v", [H * DV], F32, kind="ExternalInput")
    ones_in = nc.dram_tensor("ones_in", [P, P], F32R, kind="ExternalInput")
    ck_tab = nc.dram_tensor("ck_tab", [CH, DR // 2], F32, kind="ExternalInput")
    sk_tab = nc.dram_tensor("sk_tab", [CH, DR // 2], F32, kind="ExternalInput")
    cq_tab = nc.dram_tensor("cq_tab", [P, CH], F32, kind="ExternalInput")
    sq_tab = nc.dram_tensor("sq_tab", [P, CH], F32, kind="ExternalInput")
    out_c = nc.dram_tensor("out_c", [CH, D], F32, kind="ExternalOutput")

    # collective bounce buffers (internal DRAM)
    kv_in = nc.dram_tensor("kv_in", [5 * P, CH], BF16)
    kv_out = nc.dram_tensor("kv_out", [G * 5 * P, CH], BF16)
    kn_in = nc.dram_tensor("kn_in", [NHG * P, CH], BF16)
    kn_out = nc.dram_tensor("kn_out", [G * NHG * P, CH], BF16)
    vl_in = nc.dram_tensor("vl_in", [CH, NGG * CH], BF16)
    vl_out = nc.dram_tensor("vl_out", [G * CH, NGG * CH], BF16)

    with tile.TileContext(nc) as tc, ExitStack() as octx:
        res = octx.enter_context(tc.tile_pool(name="res", bufs=1))
        kfull = res.tile([P, 5, S], BF16)   # gathered latent^T (4) + rope^T (1)
        kupT = res.tile([P, 4, H * DN], BF16)
        vupT = res.tile([P, 4, H * DV], BF16)

        consts = octx.enter_context(tc.tile_pool(name="consts", bufs=1))
        ident = consts.tile([P, P], BF16)
        make_identity(nc, ident)
        ones_t = consts.tile([P, P], F32R)
        eps_t = consts.tile([P, 1], F32)
        nc.vector.memset(eps_t, EPS)
        cq_t = consts.tile([P, CH], F32)
        sq_t = consts.tile([P, CH], F32)
        bqn_t = consts.tile([P, H], F32)
        bqr_t = consts.tile([P, 8], F32)
        bkn_t = consts.tile([P, H], F32)
        bvv_bc = consts.tile([P, H * DV], F32)

        qstage = octx.enter_context(tc.tile_pool(name="qstage", bufs=1))
        qro_all = qstage.tile([P, G, 2, CH], BF16)
        qnope_all = qstage.tile([P, H, CH], BF16)

        wop = octx.enter_context(tc.tile_pool(name="wop", bufs=6))

        with ExitStack() as p1all:
            lnsp = p1all.enter_context(tc.tile_pool(name="lnsp", bufs=1))
            qproj = p1all.enter_context(tc.tile_pool(name="qproj", bufs=1))
            qlat_t = qproj.tile([P, NLT, CH], BF16)

            qln_all = lnsp.tile([P, NQT, LAT], BF16)
            lnf_loc = lnsp.tile([P, NKTC, R], BF16)
            kro_loc = lnsp.tile([P, NKTC, P], BF16)
            kfl = lnsp.tile([P, 5, CH], BF16)

            p1ab = p1all.enter_context(ExitStack())
            hsqp = p1ab.enter_context(tc.tile_pool(name="hsqp", bufs=1))
            wkvp = p1ab.enter_context(tc.tile_pool(name="wkvp", bufs=1))
            hsq_all = hsqp.tile([P, 2, NDT, 2, P], BF16)
            wkv_all = wkvp.tile([P, NDT, R + DR], BF16)
            ck_t = wkvp.tile([P, NKTC, DR // 2], F32)
            sk_t = wkvp.tile([P, NKTC, DR // 2], F32)

            # =============== segment B': kv-mix on own 512 keys ===============
            with ExitStack() as pB:
                mixp = pB.enter_context(tc.tile_pool(name="mixp", bufs=3))
                lnp = pB.enter_context(tc.tile_pool(name="lnp", bufs=2))
                psB = pB.enter_context(tc.tile_pool(name="psB", bufs=1, space="PSUM"))

                # interleave kv weights with own hidden-state tiles so the
                # first matmul can start after ~1.1MB of DMA
                for a in range(4):
                    nc.sync.dma_start(
                        wkv_all[:, 4 * a:4 * (a + 1), :],
                        wkva_t[512 * a:512 * (a + 1), :].rearrange(
                            "(t p) c -> p t c", p=P
                        ),
                    )
                    for pair in range(2):
                        nc.sync.dma_start(
                            hsq_all[:, pair, 4 * a:4 * (a + 1), :, :],
                            hsqt[pair, 4 * a:4 * (a + 1)].rearrange(
                                "d k p c -> p d k c"
                            ),
                        )
                nc.sync.dma_start(
                    ck_t[:], ck_tab.ap().rearrange("(t p) j -> p t j", p=P)
                )
                nc.sync.dma_start(
                    sk_t[:], sk_tab.ap().rearrange("(t p) j -> p t j", p=P)
                )

                # phase-2 weights streamed during kv-mix
                def _stream_weights(dt):
                    if dt % 4 == 1:
                        rc = dt // 4
                        nc.scalar.dma_start(
                            kupT[:, rc, :], kup_t[rc * P:(rc + 1) * P, :]
                        )
                    elif dt % 4 == 3:
                        rc = dt // 4
                        nc.scalar.dma_start(
                            vupT[:, rc, :], vup_t[rc * P:(rc + 1) * P, :]
                        )
                    elif dt == 2:
                        nc.sync.dma_start(ones_t[:], ones_in[:, :])
                        nc.sync.dma_start(
                            bkn_t[:], bkn_v.ap().rearrange("(h p) -> p h", p=P)
                        )
                        nc.sync.dma_start(bvv_bc[:], _bcast_rows(bvv_v, P, H * DV))
                    elif dt == 6:
                        nc.sync.dma_start(
                            bqn_t[:], bqn_v.ap().rearrange("(h p) -> p h", p=P)
                        )
                        nc.sync.dma_start(
                            bqr_t[:], bqr_v.ap().rearrange("(a p) -> p a", p=P)
                        )
                        nc.sync.dma_start(cq_t[:], cq_tab[:, :])
                        nc.sync.dma_start(sq_t[:], sq_tab[:, :])

                for ktp in range(2):
                    pm = [
                        psB.tile([P, 2, 512], F32, tag=f"pmix{i}", bufs=2,
                                 name=f"pm{i}")
                        for i in range(2)
                    ]
                    for a in range(4):
                        hk8 = hsq_all[:, ktp, 4 * a:4 * (a + 1), :, :]
                        for di in range(4):
                            dt = 4 * a + di
                            st = (dt == 0)
                            sp = (dt == NDT - 1)
                            for i in range(2):
                                nc.tensor.matmul(
                                    pm[i][:, 0, 0:288], hk8[:, di, i, :],
                                    wkv_all[:, dt, 0:288], start=st, stop=sp,
                                )
                                nc.tensor.matmul(
                                    pm[i][:, 1, 0:288], hk8[:, di, i, :],
                                    wkv_all[:, dt, 288:576], start=st, stop=sp,
                                )
                            if ktp == 0:
                                _stream_weights(dt)
                    for i in range(2):
                        kt = 2 * ktp + i
                        kvmix = mixp.tile([P, R + DR], F32, tag="kvmix")
                        nc.scalar.copy(kvmix[:, 0:288], pm[i][:, 0, 0:288])
                        nc.scalar.copy(kvmix[:, 288:576], pm[i][:, 1, 0:288])

                        stats = lnp.tile([P, 6], F32, tag="stats")
                        nc.vector.bn_stats(stats[:], kvmix[:, 0:R])
                        mv = lnp.tile([P, 2], F32, tag="mv")
                        nc.vector.bn_aggr(mv[:], stats[:])
                        rstd = lnp.tile([P, 1], F32, tag="rstd")
                        nc.scalar.activation(
                            rstd[:], mv[:, 1:2], AF.Sqrt, bias=eps_t[:]
                        )
                        nc.vector.reciprocal(rstd[:], rstd[:])
                        nc.vector.tensor_scalar(
                            lnf_loc[:, kt, :], kvmix[:, 0:R], mv[:, 0:1],
                            rstd[:], op0=OP.subtract, op1=OP.mult,
                        )

                        # RoPE, rotated pairs duplicated to cols 64:128
                        t1 = lnp.tile([P, DR // 2], F32, tag="t1")
                        t2 = lnp.tile([P, DR // 2], F32, tag="t2")
                        x1 = kvmix[:, R:R + 32]
                        x2 = kvmix[:, R + 32:R + 64]
                        kro = kro_loc[:, kt, :]
                        nc.vector.tensor_tensor(t2[:], x1, ck_t[:, kt, :], OP.mult)
                        nc.vector.tensor_tensor(t1[:], x2, sk_t[:, kt, :], OP.mult)
                        nc.vector.tensor_tensor(kro[:, 0:32], t2[:], t1[:], OP.subtract)
                        nc.vector.tensor_tensor(kro[:, 64:96], t2[:], t1[:], OP.subtract)
                        nc.vector.tensor_tensor(t2[:], x1, sk_t[:, kt, :], OP.mult)
                        nc.vector.tensor_tensor(t1[:], x2, ck_t[:, kt, :], OP.mult)
                        nc.vector.tensor_tensor(kro[:, 32:64], t2[:], t1[:], OP.add)
                        nc.vector.tensor_tensor(kro[:, 96:128], t2[:], t1[:], OP.add)

            # ========== segment B2a: kv transposes + AG1 (kfull planes) ==========
            with ExitStack() as pB2a:
                psCa = pB2a.enter_context(tc.tile_pool(name="psCa", bufs=1, space="PSUM"))
                for kt in range(NKTC):
                    pt = psCa.tile([P, 5, P], BF16, tag="ptr", bufs=2)
                    for j in range(4):
                        nc.tensor.transpose(
                            pt[:, j, :],
                            lnf_loc[:, kt, j * P:(j + 1) * P], ident[:],
                        )
                    nc.tensor.transpose(pt[:, 4, :], kro_loc[:, kt, :], ident[:])
                    dst = kfl[:, 0:5, kt * P:(kt + 1) * P]
                    if kt % 2 == 0:
                        nc.vector.tensor_copy(dst, pt[:])
                    else:
                        nc.scalar.copy(dst, pt[:])

                nc.scalar.dma_start(
                    kv_in.ap().rearrange("(a p) c -> p a c", p=P), kfl[:]
                )
                nc.gpsimd.collective_compute(
                    "AllGather", OP.bypass, replica_groups=RG,
                    ins=[kv_in.ap().opt()], outs=[kv_out.ap().opt()],
                )

            # ================= segment A: q-mix matmuls + LN =================
            with ExitStack() as pA:
                wqap = pA.enter_context(tc.tile_pool(name="wqap", bufs=4))
                mixp = pA.enter_context(tc.tile_pool(name="mixp", bufs=1))
                lnp = pA.enter_context(tc.tile_pool(name="lnp", bufs=2))
                psA = pA.enter_context(tc.tile_pool(name="psA", bufs=1, space="PSUM"))

                qmix_all = mixp.tile([P, NQT, LAT], BF16)
                for j in range(3):
                    pqj = psA.tile([P, NQT, 512], F32, tag="pq", bufs=2)
                    for a in range(4):
                        wqa_c = wqap.tile([P, 4, 512], BF16, tag="wqa")
                        nc.sync.dma_start(
                            wqa_c[:],
                            wqa_t[a * 512:(a + 1) * 512,
                                  j * 512:(j + 1) * 512].rearrange(
                                "(t p) c -> p t c", p=P
                            ),
                        )
                        for i in range(4):
                            dt = 4 * a + i
                            for qt in range(NQT):
                                nc.tensor.matmul(
                                    pqj[:, qt, :],
                                    hsq_all[:, qt // 2, dt, qt % 2, :],
                                    wqa_c[:, i, :],
                                    start=(dt == 0), stop=(dt == NDT - 1),
                                )
                    for qt in range(NQT):
                        nc.vector.tensor_copy(
                            qmix_all[:, qt, j * 512:(j + 1) * 512], pqj[:, qt, :]
                        )

                for qt in range(NQT):
                    statsq = lnp.tile([P, 3, 6], F32, tag="statsq")
                    for j in range(3):
                        nc.vector.bn_stats(
                            statsq[:, j, :], qmix_all[:, qt, j * 512:(j + 1) * 512]
                        )
                    mvq = lnp.tile([P, 2], F32, tag="mv")
                    nc.vector.bn_aggr(mvq[:], statsq[:])
                    rstdq = lnp.tile([P, 1], F32, tag="rstd")
                    nc.scalar.activation(
                        rstdq[:], mvq[:, 1:2], AF.Sqrt, bias=eps_t[:]
                    )
                    nc.vector.reciprocal(rstdq[:], rstdq[:])
                    nc.vector.tensor_scalar(
                        qln_all[:, qt, :], qmix_all[:, qt, :], mvq[:, 0:1],
                        rstdq[:], op0=OP.subtract, op1=OP.mult,
                    )

            # hidden states + kv weights are dead now; free before phase 1.5
            p1ab.close()

            # == segment B2b: gathered-share v_lat / k_nope on own keys + AGs ==
            # (v_lat AG first: its consumer, pv of head 8, comes before the
            # consumer of gathered k_nope, scores of head 12)
            with ExitStack() as pB2b:
                klocp = pB2b.enter_context(tc.tile_pool(name="klocp", bufs=1))
                psCb = pB2b.enter_context(tc.tile_pool(name="psCb", bufs=1, space="PSUM"))
                kng = klocp.tile([P, NHG, CH], BF16)
                vlg = klocp.tile([P, NKTC, NGG * CH], BF16)

                for kt in range(NKTC):
                    for gg in range(NGG):
                        g = G_GATH + gg
                        pvv = psCb.tile([P, CH], F32, tag="pvv", bufs=2)
                        for rc in range(4):
                            nc.tensor.matmul(
                                pvv[:], kfl[:, rc, kt * P:(kt + 1) * P],
                                vupT[:, rc, g * 512:(g + 1) * 512],
                                start=(rc == 0), stop=(rc == 3),
                            )
                        nc.vector.tensor_tensor(
                            vlg[:, kt, gg * 512:(gg + 1) * 512], pvv[:],
                            bvv_bc[:, g * 512:(g + 1) * 512], OP.add,
                        )
                nc.scalar.dma_start(
                    vl_in.ap().rearrange("(t p) c -> p t c", p=P), vlg[:]
                )
                nc.gpsimd.collective_compute(
                    "AllGather", OP.bypass, replica_groups=RG,
                    ins=[vl_in.ap().opt()], outs=[vl_out.ap().opt()],
                )

                for j in range(NHG):
                    h = H_GATH + j
                    pk = psCb.tile([P, CH], F32, tag="pk", bufs=2)
                    for rc in range(4):
                        nc.tensor.matmul(
                            pk[:], kupT[:, rc, h * P:(h + 1) * P],
                            kfl[:, rc, :],
                            start=(rc == 0), stop=(rc == 3),
                        )
                    nc.scalar.add(kng[:, j, :], pk[:], bkn_t[:, h:h + 1])
                nc.scalar.dma_start(
                    kn_in.ap().rearrange("(h p) c -> p h c", p=P), kng[:]
                )
                nc.gpsimd.collective_compute(
                    "AllGather", OP.bypass, replica_groups=RG,
                    ins=[kn_in.ap().opt()], outs=[kn_out.ap().opt()],
                )

            # ============ segment C: batched q latent transposes ============
            with ExitStack() as pC:
                psC2 = pC.enter_context(tc.tile_pool(name="psC2", bufs=1, space="PSUM"))
                ev = 0
                for qt in range(NQT):
                    for half in range(2):
                        ptq = psC2.tile([P, 6, P], BF16, tag="ptr", bufs=2)
                        for i in range(6):
                            lt = half * 6 + i
                            nc.tensor.transpose(
                                ptq[:, i, :],
                                qln_all[:, qt, lt * P:(lt + 1) * P], ident[:],
                            )
                        dst = qlat_t[:, half * 6:(half + 1) * 6,
                                     qt * P:(qt + 1) * P]
                        if ev % 2 == 0:
                            nc.vector.tensor_copy(dst, ptq[:])
                        else:
                            nc.scalar.copy(dst, ptq[:])
                        ev += 1

            # ===== phase 1.5: q_rope (4 groups) + q_nope (16 heads) staged =====
            with ExitStack() as p15:
                wqrp = p15.enter_context(tc.tile_pool(name="wqrp", bufs=1))
                wqs = p15.enter_context(tc.tile_pool(name="wqs", bufs=3))
                qwork = p15.enter_context(tc.tile_pool(name="qwork", bufs=2))
                ps15 = p15.enter_context(tc.tile_pool(name="ps15", bufs=1, space="PSUM"))

                # preload all q_rope weights, THEN the gathered kfull (which
                # waits on AG1 and would head-of-line block later DMAs)
                wqr_all = wqrp.tile([P, 8, NLT, P], BF16)
                for g in range(G):
                    for half in range(2):
                        col0 = half * 512 + g * P
                        nc.sync.dma_start(
                            wqr_all[:, half * 4 + g, :, :],
                            wqr_t[:, col0:col0 + P].rearrange(
                                "(t p) c -> p t c", p=P
                            ),
                        )
                for a in range(5):
                    nc.scalar.dma_start(
                        kfull[:, a, :],
                        kv_out.ap().rearrange(
                            "(r a p) c -> p a r c", r=G, a=5
                        )[:, a],
                    )

                for g in range(G):
                    qraw = qwork.tile([P, 2, CH], F32, tag="qraw")
                    for half in range(2):
                        pr = ps15.tile([P, 512], F32, tag="proj", bufs=2)
                        for lt in range(NLT):
                            nc.tensor.matmul(
                                pr[:], wqr_all[:, half * 4 + g, lt, :],
                                qlat_t[:, lt, :],
                                start=(lt == 0), stop=(lt == NLT - 1),
                            )
                        nc.scalar.add(
                            qraw[:, half, :], pr[:],
                            bqr_t[:, half * 4 + g:half * 4 + g + 1],
                        )
                    tm = qwork.tile([P, CH], F32, tag="tm")
                    tn = qwork.tile([P, CH], F32, tag="tn")
                    x1, x2 = qraw[:, 0, :], qraw[:, 1, :]
                    nc.vector.tensor_tensor(tm[:], x2, sq_t[:], OP.mult)
                    nc.vector.tensor_tensor(tn[:], x1, cq_t[:], OP.mult)
                    nc.vector.tensor_tensor(qro_all[:, g, 0, :], tn[:], tm[:], OP.subtract)
                    nc.vector.tensor_tensor(tm[:], x2, cq_t[:], OP.mult)
                    nc.vector.tensor_tensor(tn[:], x1, sq_t[:], OP.mult)
                    nc.vector.tensor_tensor(qro_all[:, g, 1, :], tn[:], tm[:], OP.add)

                for h in range(H):
                    wb = wqs.tile([P, NLT, P], BF16, tag="wq")
                    nc.sync.dma_start(
                        wb[:],
                        wqb_t[:, h * P:(h + 1) * P].rearrange(
                            "(t p) c -> p t c", p=P
                        ),
                    )
                    pn = ps15.tile([P, 512], F32, tag="proj", bufs=2)
                    for lt in range(NLT):
                        nc.tensor.matmul(
                            pn[:], wb[:, lt, :], qlat_t[:, lt, :],
                            start=(lt == 0), stop=(lt == NLT - 1),
                        )
                    nc.scalar.add(qnope_all[:, h, :], pn[:], bqn_t[:, h:h + 1])

        # ====================== phase 2: attention head loop ======================
        attp = octx.enter_context(tc.tile_pool(name="attp", bufs=1))
        avT = attp.tile([P, H, CH], BF16)

        wo_pre = []
        with ExitStack() as p2:
            hwork = p2.enter_context(tc.tile_pool(name="hwork", bufs=1))
            gwork = p2.enter_context(tc.tile_pool(name="gwork", bufs=2))
            probs_p = p2.enter_context(tc.tile_pool(name="probs_p", bufs=2))
            foldp = p2.enter_context(tc.tile_pool(name="foldp", bufs=3))
            ps2 = p2.enter_context(tc.tile_pool(name="ps2", bufs=1, space="PSUM"))

            def load_knope(h):
                t = hwork.tile([P, S], BF16, tag="knopeT", bufs=2)
                nc.scalar.dma_start(
                    t[:],
                    kn_out.ap().rearrange(
                        "(r h p) c -> p h r c", r=G, h=NHG
                    )[:, h - H_GATH],
                )
                return t

            def load_vlat(g):
                t = gwork.tile([P, NKT, CH], BF16, tag="vlatq", bufs=2)
                nc.scalar.dma_start(
                    t[:],
                    vl_out.ap().rearrange(
                        "(r t p) (g c) -> p g r t c", r=G, t=NKTC, g=NGG
                    )[:, g - G_GATH],
                )
                return t

            kn_pending = None
            vl_pending = None
            vlatq = None
            for h in range(H):
                g, m = divmod(h, 4)

                if m == 0:
                    if g < G_GATH:
                        # local v_lat for this group from gathered kfull
                        vlatq = gwork.tile([P, NKT, CH], BF16, tag="vlatq",
                                           bufs=2)
                        for kt in range(NKT):
                            pv1 = ps2.tile([P, 512], F32, tag="proj", bufs=2)
                            for rc in range(4):
                                nc.tensor.matmul(
                                    pv1[:], kfull[:, rc, kt * P:(kt + 1) * P],
                                    vupT[:, rc, g * 512:(g + 1) * 512],
                                    start=(rc == 0), stop=(rc == 3),
                                )
                            nc.vector.tensor_tensor(
                                vlatq[:, kt, :], pv1[:],
                                bvv_bc[:, g * 512:(g + 1) * 512], OP.add,
                            )
                    else:
                        vlatq = vl_pending

                if h == H - 1:
                    for i in range(3):
                        wo = wop.tile([P, 512], BF16, tag="wo")
                        nc.sync.dma_start(wo[:], wo_t[i * P:(i + 1) * P, 0:512])
                        wo_pre.append(wo)

                # k_nope^T for this head: local compute or gathered load
                if h < H_GATH:
                    knopeT = hwork.tile([P, S], BF16, tag="knopeT", bufs=2)
                    for kc in range(4):
                        pk = ps2.tile([P, 512], F32, tag="proj", bufs=2)
                        for rc in range(4):
                            nc.tensor.matmul(
                                pk[:], kupT[:, rc, h * P:(h + 1) * P],
                                kfull[:, rc, kc * 512:(kc + 1) * 512],
                                start=(rc == 0), stop=(rc == 3),
                            )
                        nc.scalar.add(
                            knopeT[:, kc * 512:(kc + 1) * 512], pk[:],
                            bkn_t[:, h:h + 1],
                        )
                else:
                    knopeT = kn_pending
                if h + 1 >= H_GATH and h + 1 < H:
                    kn_pending = load_knope(h + 1)

                qnope = qnope_all[:, h, :]
                qro = qro_all[:, g, :, :]
                qropeT = hwork.tile([P, CH], BF16, tag="qropeT", bufs=2)
                nc.sync.dma_start(qropeT[0:32, :], qro[m * 32:(m + 1) * 32, 0, :])
                nc.sync.dma_start(qropeT[32:64, :], qro[m * 32:(m + 1) * 32, 1, :])
                nc.sync.dma_start(qropeT[64:96, :], qro[m * 32:(m + 1) * 32, 0, :])
                nc.sync.dma_start(qropeT[96:128, :], qro[m * 32:(m + 1) * 32, 1, :])

                # prefetch next group's gathered v_lat late (after AG3 lands)
                # so the waiting DMA doesn't head-of-line-block this queue
                if m == 2 and g + 1 >= G_GATH and g + 1 < G:
                    vl_pending = load_vlat(g + 1)

                probs = probs_p.tile([P, NKT, CH], BF16, tag="probs")
                folds = []
                quads = []
                octs = []
                pv = ps2.tile([P, 512], F32, tag="attn", bufs=1)
                pd = ps2.tile([P, 512], F32, tag="den", bufs=1)
                for p in range(NKT // 2):
                    kt, kt1 = 2 * p, 2 * p + 1
                    sc = ps2.tile([P, 2, 512], F32, tag="scores", bufs=2)
                    nc.tensor.matmul(
                        sc[:, 0, :], knopeT[:, kt * P:(kt + 1) * P], qnope,
                        start=True, stop=False,
                    )
                    nc.tensor.matmul(
                        sc[:, 1, :], knopeT[:, kt1 * P:(kt1 + 1) * P], qnope,
                        start=True, stop=False,
                    )
                    nc.tensor.matmul(
                        sc[:, 0, :], kfull[0:DR, 4, kt * P:(kt + 1) * P],
                        qropeT[0:DR, :], start=False, stop=True,
                    )
                    nc.tensor.matmul(
                        sc[:, 1, :], kfull[DR:P, 4, kt1 * P:(kt1 + 1) * P],
                        qropeT[DR:P, :], start=False, stop=True,
                        tile_position=(DR, 0),
                    )
                    nc.scalar.activation(probs[:, kt:kt + 2, :], sc[:], AF.Exp)
                    ft = foldp.tile([P, CH], F32R, tag="fold")
                    nc.vector.tensor_tensor(
                        ft[:], probs[:, kt, :], probs[:, kt1, :], OP.add
                    )
                    folds.append(ft)
                    if p % 2 == 1:
                        fq = foldp.tile([P, CH], F32R, tag="foldq")
                        nc.vector.tensor_tensor(
                            fq[:], folds[p - 1][:], folds[p][:], OP.add
                        )
                        quads.append(fq)
                    if p % 4 == 3:
                        fo = foldp.tile([P, CH], F32R, tag="foldo")
                        nc.vector.tensor_tensor(
                            fo[:], quads[-2][:], quads[-1][:], OP.add
                        )
                        octs.append(fo)
                    if p >= 1:
                        nc.tensor.matmul(
                            pv[:], vlatq[:, kt - 2, m * P:(m + 1) * P],
                            probs[:, kt - 2, :], start=(p == 1), stop=False,
                        )
                        nc.tensor.matmul(
                            pv[:], vlatq[:, kt - 1, m * P:(m + 1) * P],
                            probs[:, kt - 1, :], start=False, stop=False,
                        )
                    if p == 5:
                        nc.tensor.matmul(
                            pd[:], ones_t[:], octs[0][:],
                            start=True, stop=False,
                        )
                nc.tensor.matmul(
                    pv[:], vlatq[:, NKT - 2, m * P:(m + 1) * P],
                    probs[:, NKT - 2, :], start=False, stop=False,
                )
                nc.tensor.matmul(
                    pv[:], vlatq[:, NKT - 1, m * P:(m + 1) * P],
                    probs[:, NKT - 1, :], start=False, stop=True,
                )
                nc.tensor.matmul(
                    pd[:], ones_t[:], octs[1][:], start=False, stop=True,
                )
                recip = hwork.tile([P, CH], F32, tag="recip", bufs=2)
                nc.vector.reciprocal_approx_fast(recip[:], pd[:])
                nc.vector.tensor_tensor(avT[:, h, :], pv[:], recip[:], OP.mult)

        # ================== phase 3: o_proj in quarter passes ==================
        with ExitStack() as p3:
            outp = p3.enter_context(tc.tile_pool(name="outp", bufs=4))
            ps3 = p3.enter_context(tc.tile_pool(name="ps3", bufs=1, space="PSUM"))

            pre = wo_pre
            for quarter in range(4):
                po = ps3.tile([P, NQT, 512], F32, tag="po", bufs=2)
                for kt in range(H):
                    if kt < len(pre):
                        wo = pre[kt]
                    else:
                        wo = wop.tile([P, 512], BF16, tag="wo")
                        nc.sync.dma_start(
                            wo[:],
                            wo_t[kt * P:(kt + 1) * P,
                                 quarter * 512:(quarter + 1) * 512],
                        )
                    for qc in range(NQT):
                        nc.tensor.matmul(
                            po[:, qc, :],
                            avT[:, kt, qc * P:(qc + 1) * P],
                            wo[:],
                            start=(kt == 0), stop=(kt == H - 1),
                        )
                pre = []
                if quarter < 3:
                    for i in range(2):
                        wo = wop.tile([P, 512], BF16, tag="wo")
                        nc.sync.dma_start(
                            wo[:],
                            wo_t[i * P:(i + 1) * P,
                                 (quarter + 1) * 512:(quarter + 2) * 512],
                        )
                        pre.append(wo)
                for qc in range(NQT):
                    ot = outp.tile([P, 512], F32, tag="ot")
                    if qc % 2 == 0:
                        nc.vector.tensor_copy(ot[:], po[:, qc, :])
                    else:
                        nc.scalar.copy(ot[:], po[:, qc, :])
                    nc.sync.dma_start(
                        out_c[
                            qc * P:(qc + 1) * P,
                            quarter * 512:(quarter + 1) * 512,
                        ],
                        ot[:],
                    )

    nc.compile()
    return nc


_NC_CACHE = None


def _get_nc():
    global _NC_CACHE
    if _NC_CACHE is None:
        _NC_CACHE = build_nc()
    return _NC_CACHE


def _prep_in_maps(inputs):
    hidden = np.asarray(inputs["hidden_states"], dtype=np.float32)
    w_qa = np.asarray(inputs["w_qa"], dtype=np.float32)
    ln_qa_g = np.asarray(inputs["ln_qa_g"], dtype=np.float32)
    ln_qa_b = np.asarray(inputs["ln_qa_b"], dtype=np.float32)
    w_qb = np.asarray(inputs["w_qb"], dtype=np.float32)
    w_qrope = np.asarray(inputs["w_qrope"], dtype=np.float32)
    w_kva = np.asarray(inputs["w_kva"], dtype=np.float32)
    ln_kva_g = np.asarray(inputs["ln_kva_g"], dtype=np.float32)
    ln_kva_b = np.asarray(inputs["ln_kva_b"], dtype=np.float32)
    w_kvb = np.asarray(inputs["w_kvb"], dtype=np.float32)
    w_o = np.asarray(inputs["w_o"], dtype=np.float32)
    pos = np.asarray(inputs["position_ids"]).astype(np.int64)

    bf = bfloat16
    hidden_b = hidden.astype(bf)
    hst_all = [
        hidden_b[b].T.reshape(NDT, P, NKT // 2, 2, P).transpose(2, 0, 3, 1, 4)
        for b in range(B)
    ]
    wqa_t = np.ascontiguousarray(w_qa.T.astype(bf))
    # LN gamma folded into q up-projections; beta becomes an output bias:
    # q_nope = (ln0*g + b) @ w_qb.T = ln0 @ (w_qb*g).T + w_qb @ b
    wqb_g = w_qb * ln_qa_g[None, :]
    bqn = (w_qb @ ln_qa_b).astype(np.float32)
    wqb_t = np.ascontiguousarray(wqb_g.T.astype(bf))
    wqr_s = SCALE * w_qrope
    bqr_full = (wqr_s @ ln_qa_b).astype(np.float32)
    wqr_g = (wqr_s * ln_qa_g[None, :]).T
    wqr_t = np.ascontiguousarray(
        wqr_g.reshape(LAT, H, 2, DR // 2).transpose(0, 2, 1, 3)
        .reshape(LAT, H * DR).astype(bf)
    )
    bqr_perm = np.ascontiguousarray(
        bqr_full.reshape(H, 2, DR // 2).transpose(1, 0, 2).reshape(H * DR)
    )
    wkva_t = np.ascontiguousarray(w_kva.T.astype(bf))
    kup = (SCALE * w_kvb[: H * DN]).reshape(H, DN, R)
    bkn = (kup @ ln_kva_b).reshape(H * DN).astype(np.float32)
    kup_g = kup * ln_kva_g[None, None, :]
    kup_t = np.ascontiguousarray(
        kup_g.transpose(2, 0, 1).reshape(R, H * DN).astype(bf)
    )
    vup = w_kvb[H * DN:].reshape(H, DV, R)
    bvv = (vup @ ln_kva_b).reshape(H * DV).astype(np.float32)
    vup_g = vup * ln_kva_g[None, None, :]
    vup_t = np.ascontiguousarray(
        vup_g.transpose(2, 0, 1).reshape(R, H * DV).astype(bf)
    )
    wo_t = np.ascontiguousarray(w_o.T.astype(bf))
    ones_in = np.ones((P, P), dtype=np.float32)

    inv_freq = 1.0 / (10000.0 ** (np.arange(0, DR, 2, dtype=np.float64) / DR))
    ang = pos[:, None].astype(np.float64) * inv_freq[None, :]
    cosf = np.ascontiguousarray(np.cos(ang).astype(np.float32))
    sinf = np.ascontiguousarray(np.sin(ang).astype(np.float32))

    in_maps = []
    for c in range(N_CORES):
        b, ch = divmod(c, NQT)
        qs = ch * CH
        cq = np.ascontiguousarray(np.tile(cosf[qs:qs + CH, :].T, (NQT, 1)))
        sq = np.ascontiguousarray(np.tile(sinf[qs:qs + CH, :].T, (NQT, 1)))
        myp = [2 * ch, 2 * ch + 1]
        in_maps.append({
            "hsqt": np.ascontiguousarray(hst_all[b][myp]),
            "wqa_t": wqa_t,
            "wqb_t": wqb_t,
            "wqr_t": wqr_t,
            "wkva_t": wkva_t,
            "kup_t": kup_t,
            "vup_t": vup_t,
            "wo_t": wo_t,
            "bqn_v": bqn,
            "bqr_v": bqr_perm,
            "bkn_v": bkn,
            "bvv_v": bvv,
            "ones_in": ones_in,
            "ck_tab": np.ascontiguousarray(cosf[qs:qs + CH]),
            "sk_tab": np.ascontiguousarray(sinf[qs:qs + CH]),
            "cq_tab": cq,
            "sq_tab": sq,
        })
    return in_maps


def kernel(**inputs) -> np.ndarray:
    nc = _get_nc()
    in_maps = _prep_in_maps(inputs)
    res = run_bass_kernel_spmd(nc, in_maps, core_ids=list(range(N_CORES)))
    out = np.empty((B, S, D), dtype=np.float32)
    for c in range(N_CORES):
        b, ch = divmod(c, NQT)
        out[b, ch * CH:(ch + 1) * CH, :] = res.results[c]["out_c"]
    return out


# revision 14
# speedup vs baseline: 1.1570x; 1.0566x over previous
"""MLA (multi-head latent attention) prefill kernel for 8 Trainium2 NeuronCores.

Sharding: data-parallel over (batch, query-chunk) for the q path, attention
and o_proj. The KV path is split between recompute and collectives, sized to
the measured ~55 GB/s 4-rank AllGather rate (replica groups [[0,1,2,3],
[4,5,6,7]], one per batch):
 - kv-mix runs only over the core's own 512 keys (the same hidden rows as its
   q chunk, so no cross-chunk hidden-state DMAs at all); the resulting
   latent+rope planes (0.65MB) are AllGathered into the full kfull (AG1).
 - k_nope for heads 0-7 and v_lat for head-groups 0-1 are recomputed per-core
   from kfull inside the head loop (baseline style).
 - k_nope for heads 8-15 and v_lat for groups 2-3 are computed once per core
   over its own 512 keys, AllGathered (AG2/AG3, 4MB each) while the PE chews
   through the q path, and DMA-loaded just-in-time in the head loop.
Collectives run on TOPSP/SDMA silicon and overlap with PE compute; AG2/AG3
have >100us of slack before their consumers. Per-core matmul work drops from
~37.6 GF to ~30 GF with zero PE stalls by construction.

Structure:
 - Segment B': kv-mix over own keys, LN + RoPE, transposes -> kfl (5 planes),
   evict + AG1; local k_nope[h8..15] / v_lat[g2..3] shares -> AG2 / AG3.
 - Segment A: q-mix matmuls + LayerNorm (unchanged from baseline).
 - Segment C: q latent transposes (batched, 6/bank).
 - Phase 1.5: q_rope for all 4 head groups + q_nope for all 16 heads staged
   in SBUF; the gathered kfull is DMA-loaded here (after AG1).
 - Phase 2 head loop: per head, scores contract 128+64 channels; softmax
   denominator via pair->quad->oct folds + two ones-matmuls; attn @ v_lat.
 - Phase 3: o_proj in 4 quarter passes with double-buffered PSUM.
"""

import math
from contextlib import ExitStack

import numpy as np
from ml_dtypes import bfloat16

import concourse.bass as bass
import concourse.tile as tile
from concourse import bacc, mybir
from concourse.bass_utils import run_bass_kernel_spmd
from concourse.masks import make_identity

F32 = mybir.dt.float32
F32R = mybir.dt.float32r
BF16 = mybir.dt.bfloat16
AF = mybir.ActivationFunctionType
OP = mybir.AluOpType

B, S, D = 2, 2048, 2048
H = 16
LAT = 1536
R = 512
DN, DR, DV = 128, 64, 128
EPS = 1e-5
SCALE = 1.0 / math.sqrt(DN + DR)

P = 128
CH = 512
NQT = CH // P      # 4 q tiles per chunk
NKT = S // P       # 16 key tiles total
NKTC = CH // P     # 4 key tiles in own chunk
NDT = D // P
NLT = LAT // P

N_CORES = 8
G = 4              # ranks per replica group (one batch)
RG = [[0, 1, 2, 3], [4, 5, 6, 7]]
H_GATH = 12        # heads [H_GATH..H) use gathered k_nope
G_GATH = 2         # groups [G_GATH..4) use gathered v_lat
NHG = H - H_GATH
NGG = 4 - G_GATH


def _bcast_rows(t, n, length):
    return bass.AP(tensor=t, offset=0, ap=[[0, n], [1, length]])


def build_nc():
    nc = bacc.Bacc(None, target_bir_lowering=False, num_devices=N_CORES)

    hsqt = nc.dram_tensor("hsqt", [2, NDT, 2, P, P], BF16, kind="ExternalInput")
    wqa_t = nc.dram_tensor("wqa_t", [D, LAT], BF16, kind="ExternalInput")
    wqb_t = nc.dram_tensor("wqb_t", [LAT, H * DN], BF16, kind="ExternalInput")
    wqr_t = nc.dram_tensor("wqr_t", [LAT, H * DR], BF16, kind="ExternalInput")
    wkva_t = nc.dram_tensor("wkva_t", [D, R + DR], BF16, kind="ExternalInput")
    kup_t = nc.dram_tensor("kup_t", [R, H * DN], BF16, kind="ExternalInput")
    vup_t = nc.dram_tensor("vup_t", [R, H * DV], BF16, kind="ExternalInput")
    wo_t = nc.dram_tensor("wo_t", [H * DV, D], BF16, kind="ExternalInput")
    bqn_v = nc.dram_tensor("bqn_v", [H * DN], F32, kind="ExternalInput")
    bqr_v = nc.dram_tensor("bqr_v", [H * DR], F32, kind="ExternalInput")
    bkn_v = nc.dram_tensor("bkn_v", [H * DN], F32, kind="ExternalInput")
    bvv_v = nc.dram_tensor("bvv_v", [H * DV], F32, kind="ExternalInput")
    ones_in = nc.dram_tensor("ones_in", [P, P], F32R, kind="ExternalInput")
    ck_tab = nc.dram_tensor("ck_tab", [CH, DR // 2], F32, kind="ExternalInput")
    sk_tab = nc.dram_tensor("sk_tab", [CH, DR // 2], F32, kind="ExternalInput")
    cq_tab = nc.dram_tensor("cq_tab", [P, CH], F32, kind="ExternalInput")
    sq_tab = nc.dram_tensor("sq_tab", [P, CH], F32, kind="ExternalInput")
    out_c = nc.dram_tensor("out_c", [CH, D], F32, kind="ExternalOutput")

    # collective bounce buffers (internal DRAM)
    kv_in = nc.dram_tensor("kv_in", [5 * P, CH], BF16)
    kv_out = nc.dram_tensor("kv_out", [G * 5 * P, CH], BF16)
    kn_in = nc.dram_tensor("kn_in", [NHG * P, CH], BF16)
    kn_out = nc.dram_tensor("kn_out", [G * NHG * P, CH], BF16)
    vl_in = nc.dram_tensor("vl_in", [CH, NGG * CH], BF16)
    vl_out = nc.dram_tensor("vl_out", [G * CH, NGG * CH], BF16)

    with tile.TileContext(nc) as tc, ExitStack() as octx:
        res = octx.enter_context(tc.tile_pool(name="res", bufs=1))
        kfull = res.tile([P, 5, S], BF16)   # gathered latent^T (4) + rope^T (1)
        kupT = res.tile([P, 4, H * DN], BF16)
        vupT = res.tile([P, 4, H * DV], BF16)

        consts = octx.enter_context(tc.tile_pool(name="consts", bufs=1))
        ident = consts.tile([P, P], BF16)
        make_identity(nc, ident)
        ones_t = consts.tile([P, P], F32R)
        eps_t = consts.tile([P, 1], F32)
        nc.vector.memset(eps_t, EPS)
        cq_t = consts.tile([P, CH], F32)
        sq_t = consts.tile([P, CH], F32)
        bqn_t = consts.tile([P, H], F32)
        bqr_t = consts.tile([P, 8], F32)
        bkn_t = consts.tile([P, H], F32)
        bvv_bc = consts.tile([P, H * DV], F32)

        wop = octx.enter_context(tc.tile_pool(name="wop", bufs=6))

        qproj = octx.enter_context(tc.tile_pool(name="qproj", bufs=1))
        qlat_t = qproj.tile([P, NLT, CH], BF16)

        with ExitStack() as p1all:
            lnsp = p1all.enter_context(tc.tile_pool(name="lnsp", bufs=1))

            qln_all = lnsp.tile([P, NQT, LAT], BF16)
            lnf_loc = lnsp.tile([P, NKTC, R], BF16)
            kro_loc = lnsp.tile([P, NKTC, P], BF16)
            kfl = lnsp.tile([P, 5, CH], BF16)

            p1ab = p1all.enter_context(ExitStack())
            hsqp = p1ab.enter_context(tc.tile_pool(name="hsqp", bufs=1))
            wkvp = p1ab.enter_context(tc.tile_pool(name="wkvp", bufs=1))
            hsq_all = hsqp.tile([P, 2, NDT, 2, P], BF16)
            wkv_all = wkvp.tile([P, NDT, R + DR], BF16)
            ck_t = wkvp.tile([P, NKTC, DR // 2], F32)
            sk_t = wkvp.tile([P, NKTC, DR // 2], F32)

            # =============== segment B': kv-mix on own 512 keys ===============
            with ExitStack() as pB:
                mixp = pB.enter_context(tc.tile_pool(name="mixp", bufs=3))
                lnp = pB.enter_context(tc.tile_pool(name="lnp", bufs=2))
                psB = pB.enter_context(tc.tile_pool(name="psB", bufs=1, space="PSUM"))

                # interleave kv weights with own hidden-state tiles so the
                # first matmul can start after ~1.1MB of DMA
                for a in range(4):
                    nc.sync.dma_start(
                        wkv_all[:, 4 * a:4 * (a + 1), :],
                        wkva_t[512 * a:512 * (a + 1), :].rearrange(
                            "(t p) c -> p t c", p=P
                        ),
                    )
                    for pair in range(2):
                        nc.sync.dma_start(
                            hsq_all[:, pair, 4 * a:4 * (a + 1), :, :],
                            hsqt[pair, 4 * a:4 * (a + 1)].rearrange(
                                "d k p c -> p d k c"
                            ),
                        )
                nc.sync.dma_start(
                    ck_t[:], ck_tab.ap().rearrange("(t p) j -> p t j", p=P)
                )
                nc.sync.dma_start(
                    sk_t[:], sk_tab.ap().rearrange("(t p) j -> p t j", p=P)
                )

                # phase-2 weights streamed during kv-mix
                def _stream_weights(dt):
                    if dt % 4 == 1:
                        rc = dt // 4
                        nc.scalar.dma_start(
                            kupT[:, rc, :], kup_t[rc * P:(rc + 1) * P, :]
                        )
                    elif dt % 4 == 3:
                        rc = dt // 4
                        nc.scalar.dma_start(
                            vupT[:, rc, :], vup_t[rc * P:(rc + 1) * P, :]
                        )
                    elif dt == 2:
                        nc.sync.dma_start(ones_t[:], ones_in[:, :])
                        nc.sync.dma_start(
                            bkn_t[:], bkn_v.ap().rearrange("(h p) -> p h", p=P)
                        )
                        nc.sync.dma_start(bvv_bc[:], _bcast_rows(bvv_v, P, H * DV))
                    elif dt == 6:
                        nc.sync.dma_start(
                            bqn_t[:], bqn_v.ap().rearrange("(h p) -> p h", p=P)
                        )
                        nc.sync.dma_start(
                            bqr_t[:], bqr_v.ap().rearrange("(a p) -> p a", p=P)
                        )
                        nc.sync.dma_start(cq_t[:], cq_tab[:, :])
                        nc.sync.dma_start(sq_t[:], sq_tab[:, :])

                for ktp in range(2):
                    pm = [
                        psB.tile([P, 2, 512], F32, tag=f"pmix{i}", bufs=2,
                                 name=f"pm{i}")
                        for i in range(2)
                    ]
                    for a in range(4):
                        hk8 = hsq_all[:, ktp, 4 * a:4 * (a + 1), :, :]
                        for di in range(4):
                            dt = 4 * a + di
                            st = (dt == 0)
                            sp = (dt == NDT - 1)
                            for i in range(2):
                                nc.tensor.matmul(
                                    pm[i][:, 0, 0:288], hk8[:, di, i, :],
                                    wkv_all[:, dt, 0:288], start=st, stop=sp,
                                )
                                nc.tensor.matmul(
                                    pm[i][:, 1, 0:288], hk8[:, di, i, :],
                                    wkv_all[:, dt, 288:576], start=st, stop=sp,
                                )
                            if ktp == 0:
                                _stream_weights(dt)
                    for i in range(2):
                        kt = 2 * ktp + i
                        kvmix = mixp.tile([P, R + DR], F32, tag="kvmix")
                        nc.scalar.copy(kvmix[:, 0:288], pm[i][:, 0, 0:288])
                        nc.scalar.copy(kvmix[:, 288:576], pm[i][:, 1, 0:288])

                        stats = lnp.tile([P, 6], F32, tag="stats")
                        nc.vector.bn_stats(stats[:], kvmix[:, 0:R])
                        mv = lnp.tile([P, 2], F32, tag="mv")
                        nc.vector.bn_aggr(mv[:], stats[:])
                        rstd = lnp.tile([P, 1], F32, tag="rstd")
                        nc.scalar.activation(
                            rstd[:], mv[:, 1:2], AF.Sqrt, bias=eps_t[:]
                        )
                        nc.vector.reciprocal(rstd[:], rstd[:])
                        nc.vector.tensor_scalar(
                            lnf_loc[:, kt, :], kvmix[:, 0:R], mv[:, 0:1],
                            rstd[:], op0=OP.subtract, op1=OP.mult,
                        )

                        # RoPE, rotated pairs duplicated to cols 64:128
                        t1 = lnp.tile([P, DR // 2], F32, tag="t1")
                        t2 = lnp.tile([P, DR // 2], F32, tag="t2")
                        x1 = kvmix[:, R:R + 32]
                        x2 = kvmix[:, R + 32:R + 64]
                        kro = kro_loc[:, kt, :]
                        nc.vector.tensor_tensor(t2[:], x1, ck_t[:, kt, :], OP.mult)
                        nc.vector.tensor_tensor(t1[:], x2, sk_t[:, kt, :], OP.mult)
                        nc.vector.tensor_tensor(kro[:, 0:32], t2[:], t1[:], OP.subtract)
                        nc.vector.tensor_tensor(kro[:, 64:96], t2[:], t1[:], OP.subtract)
                        nc.vector.tensor_tensor(t2[:], x1, sk_t[:, kt, :], OP.mult)
                        nc.vector.tensor_tensor(t1[:], x2, ck_t[:, kt, :], OP.mult)
                        nc.vector.tensor_tensor(kro[:, 32:64], t2[:], t1[:], OP.add)
                        nc.vector.tensor_tensor(kro[:, 96:128], t2[:], t1[:], OP.add)

            # ========== segment B2a: kv transposes + AG1 (kfull planes) ==========
            with ExitStack() as pB2a:
                psCa = pB2a.enter_context(tc.tile_pool(name="psCa", bufs=1, space="PSUM"))
                for kt in range(NKTC):
                    pt = psCa.tile([P, 5, P], BF16, tag="ptr", bufs=2)
                    for j in range(4):
                        nc.tensor.transpose(
                            pt[:, j, :],
                            lnf_loc[:, kt, j * P:(j + 1) * P], ident[:],
                        )
                    nc.tensor.transpose(pt[:, 4, :], kro_loc[:, kt, :], ident[:])
                    dst = kfl[:, 0:5, kt * P:(kt + 1) * P]
                    if kt % 2 == 0:
                        nc.vector.tensor_copy(dst, pt[:])
                    else:
                        nc.scalar.copy(dst, pt[:])

                nc.scalar.dma_start(
                    kv_in.ap().rearrange("(a p) c -> p a c", p=P), kfl[:]
                )
                nc.gpsimd.collective_compute(
                    "AllGather", OP.bypass, replica_groups=RG,
                    ins=[kv_in.ap().opt()], outs=[kv_out.ap().opt()],
                )

            # ================= segment A: q-mix matmuls + LN =================
            with ExitStack() as pA:
                wqap = pA.enter_context(tc.tile_pool(name="wqap", bufs=4))
                mixp = pA.enter_context(tc.tile_pool(name="mixp", bufs=1))
                lnp = pA.enter_context(tc.tile_pool(name="lnp", bufs=2))
                psA = pA.enter_context(tc.tile_pool(name="psA", bufs=1, space="PSUM"))

                qmix_all = mixp.tile([P, NQT, LAT], BF16)
                for j in range(3):
                    pqj = psA.tile([P, NQT, 512], F32, tag="pq", bufs=2)
                    for a in range(4):
                        wqa_c = wqap.tile([P, 4, 512], BF16, tag="wqa")
                        nc.sync.dma_start(
                            wqa_c[:],
                            wqa_t[a * 512:(a + 1) * 512,
                                  j * 512:(j + 1) * 512].rearrange(
                                "(t p) c -> p t c", p=P
                            ),
                        )
                        for i in range(4):
                            dt = 4 * a + i
                            for qt in range(NQT):
                                nc.tensor.matmul(
                                    pqj[:, qt, :],
                                    hsq_all[:, qt // 2, dt, qt % 2, :],
                                    wqa_c[:, i, :],
                                    start=(dt == 0), stop=(dt == NDT - 1),
                                )
                    for qt in range(NQT):
                        nc.vector.tensor_copy(
                            qmix_all[:, qt, j * 512:(j + 1) * 512], pqj[:, qt, :]
                        )

                for qt in range(NQT):
                    statsq = lnp.tile([P, 3, 6], F32, tag="statsq")
                    for j in range(3):
                        nc.vector.bn_stats(
                            statsq[:, j, :], qmix_all[:, qt, j * 512:(j + 1) * 512]
                        )
                    mvq = lnp.tile([P, 2], F32, tag="mv")
                    nc.vector.bn_aggr(mvq[:], statsq[:])
                    rstdq = lnp.tile([P, 1], F32, tag="rstd")
                    nc.scalar.activation(
                        rstdq[:], mvq[:, 1:2], AF.Sqrt, bias=eps_t[:]
                    )
                    nc.vector.reciprocal(rstdq[:], rstdq[:])
                    nc.vector.tensor_scalar(
                        qln_all[:, qt, :], qmix_all[:, qt, :], mvq[:, 0:1],
                        rstdq[:], op0=OP.subtract, op1=OP.mult,
                    )

            # hidden states + kv weights are dead now; free before phase 1.5
            p1ab.close()

            # gathered kfull planes (Activation queue; waits on AG1)
            for a in range(5):
                nc.scalar.dma_start(
                    kfull[:, a, :],
                    kv_out.ap().rearrange(
                        "(r a p) c -> p a r c", r=G, a=5
                    )[:, a],
                )

            # == segment B2b: gathered-share v_lat / k_nope on own keys + AGs ==
            # (v_lat AG first: its consumer, pv of head 8, comes before the
            # consumer of gathered k_nope, scores of head 12)
            with ExitStack() as pB2b:
                klocp = pB2b.enter_context(tc.tile_pool(name="klocp", bufs=1))
                psCb = pB2b.enter_context(tc.tile_pool(name="psCb", bufs=1, space="PSUM"))
                kng = klocp.tile([P, NHG, CH], BF16)
                vlg = klocp.tile([P, NKTC, NGG * CH], BF16)

                for kt in range(NKTC):
                    for gg in range(NGG):
                        g = G_GATH + gg
                        pvv = psCb.tile([P, CH], F32, tag="pvv", bufs=2)
                        for rc in range(4):
                            nc.tensor.matmul(
                                pvv[:], kfl[:, rc, kt * P:(kt + 1) * P],
                                vupT[:, rc, g * 512:(g + 1) * 512],
                                start=(rc == 0), stop=(rc == 3),
                            )
                        nc.vector.tensor_tensor(
                            vlg[:, kt, gg * 512:(gg + 1) * 512], pvv[:],
                            bvv_bc[:, g * 512:(g + 1) * 512], OP.add,
                        )
                nc.scalar.dma_start(
                    vl_in.ap().rearrange("(t p) c -> p t c", p=P), vlg[:]
                )
                nc.gpsimd.collective_compute(
                    "AllGather", OP.bypass, replica_groups=RG,
                    ins=[vl_in.ap().opt()], outs=[vl_out.ap().opt()],
                )

                for j in range(NHG):
                    h = H_GATH + j
                    pk = psCb.tile([P, CH], F32, tag="pk", bufs=2)
                    for rc in range(4):
                        nc.tensor.matmul(
                            pk[:], kupT[:, rc, h * P:(h + 1) * P],
                            kfl[:, rc, :],
                            start=(rc == 0), stop=(rc == 3),
                        )
                    nc.scalar.add(kng[:, j, :], pk[:], bkn_t[:, h:h + 1])
                nc.scalar.dma_start(
                    kn_in.ap().rearrange("(h p) c -> p h c", p=P), kng[:]
                )
                nc.gpsimd.collective_compute(
                    "AllGather", OP.bypass, replica_groups=RG,
                    ins=[kn_in.ap().opt()], outs=[kn_out.ap().opt()],
                )

            # ============ segment C: batched q latent transposes ============
            with ExitStack() as pC:
                psC2 = pC.enter_context(tc.tile_pool(name="psC2", bufs=1, space="PSUM"))
                ev = 0
                for qt in range(NQT):
                    for half in range(2):
                        ptq = psC2.tile([P, 6, P], BF16, tag="ptr", bufs=2)
                        for i in range(6):
                            lt = half * 6 + i
                            nc.tensor.transpose(
                                ptq[:, i, :],
                                qln_all[:, qt, lt * P:(lt + 1) * P], ident[:],
                            )
                        dst = qlat_t[:, half * 6:(half + 1) * 6,
                                     qt * P:(qt + 1) * P]
                        if ev % 2 == 0:
                            nc.vector.tensor_copy(dst, ptq[:])
                        else:
                            nc.scalar.copy(dst, ptq[:])
                        ev += 1

        # ====================== phase 2: attention head loop ======================
        attp = octx.enter_context(tc.tile_pool(name="attp", bufs=1))
        avT = attp.tile([P, H, CH], BF16)

        wo_pre = []
        with ExitStack() as p2:
            wqs = p2.enter_context(tc.tile_pool(name="wqs", bufs=2))
            qwork = p2.enter_context(tc.tile_pool(name="qwork", bufs=1))
            hwork = p2.enter_context(tc.tile_pool(name="hwork", bufs=2))
            gwork = p2.enter_context(tc.tile_pool(name="gwork", bufs=2))
            probs_p = p2.enter_context(tc.tile_pool(name="probs_p", bufs=2))
            foldp = p2.enter_context(tc.tile_pool(name="foldp", bufs=2))
            ps2 = p2.enter_context(tc.tile_pool(name="ps2", bufs=1, space="PSUM"))

            def load_knope(h):
                t = hwork.tile([P, S], BF16, tag="knopeT")
                nc.scalar.dma_start(
                    t[:],
                    kn_out.ap().rearrange(
                        "(r h p) c -> p h r c", r=G, h=NHG
                    )[:, h - H_GATH],
                )
                return t

            def load_vlat(g):
                t = gwork.tile([P, NKT, CH], BF16, tag="vlatq")
                nc.scalar.dma_start(
                    t[:],
                    vl_out.ap().rearrange(
                        "(r t p) (g c) -> p g r t c", r=G, t=NKTC, g=NGG
                    )[:, g - G_GATH],
                )
                return t

            kn_pending = None
            vl_pending = None
            vlatq = None
            qro = None
            for h in range(H):
                g, m = divmod(h, 4)

                if m == 0:
                    # q_rope projection + rotation for this group
                    qraw = qwork.tile([P, 2, CH], F32, tag="qraw")
                    for half in range(2):
                        wrc = wqs.tile([P, NLT, P], BF16, tag="wq")
                        col0 = half * 512 + g * P
                        nc.sync.dma_start(
                            wrc[:],
                            wqr_t[:, col0:col0 + P].rearrange(
                                "(t p) c -> p t c", p=P
                            ),
                        )
                        pr = ps2.tile([P, 512], F32, tag="proj", bufs=2)
                        for lt in range(NLT):
                            nc.tensor.matmul(
                                pr[:], wrc[:, lt, :], qlat_t[:, lt, :],
                                start=(lt == 0), stop=(lt == NLT - 1),
                            )
                        nc.scalar.add(
                            qraw[:, half, :], pr[:],
                            bqr_t[:, half * 4 + g:half * 4 + g + 1],
                        )
                    qro = qwork.tile([P, 2, CH], BF16, tag="qro")
                    tm = qwork.tile([P, CH], F32, tag="tm")
                    tn = qwork.tile([P, CH], F32, tag="tn")
                    x1, x2 = qraw[:, 0, :], qraw[:, 1, :]
                    nc.vector.tensor_tensor(tm[:], x2, sq_t[:], OP.mult)
                    nc.vector.tensor_tensor(tn[:], x1, cq_t[:], OP.mult)
                    nc.vector.tensor_tensor(qro[:, 0, :], tn[:], tm[:], OP.subtract)
                    nc.vector.tensor_tensor(tm[:], x2, cq_t[:], OP.mult)
                    nc.vector.tensor_tensor(tn[:], x1, sq_t[:], OP.mult)
                    nc.vector.tensor_tensor(qro[:, 1, :], tn[:], tm[:], OP.add)

                    # v_lat for this group: local compute or gathered load
                    if g < G_GATH:
                        vlatq = gwork.tile([P, NKT, CH], BF16, tag="vlatq")
                        for kt in range(NKT):
                            pv1 = ps2.tile([P, 512], F32, tag="proj", bufs=2)
                            for rc in range(4):
                                nc.tensor.matmul(
                                    pv1[:], kfull[:, rc, kt * P:(kt + 1) * P],
                                    vupT[:, rc, g * 512:(g + 1) * 512],
                                    start=(rc == 0), stop=(rc == 3),
                                )
                            nc.vector.tensor_tensor(
                                vlatq[:, kt, :], pv1[:],
                                bvv_bc[:, g * 512:(g + 1) * 512], OP.add,
                            )
                    else:
                        vlatq = vl_pending

                if h == H - 1:
                    for i in range(3):
                        wo = wop.tile([P, 512], BF16, tag="wo")
                        nc.sync.dma_start(wo[:], wo_t[i * P:(i + 1) * P, 0:512])
                        wo_pre.append(wo)

                # q_nope for this head
                wb = wqs.tile([P, NLT, P], BF16, tag="wq")
                nc.sync.dma_start(
                    wb[:],
                    wqb_t[:, h * P:(h + 1) * P].rearrange("(t p) c -> p t c", p=P),
                )
                pn = ps2.tile([P, 512], F32, tag="proj", bufs=2)
                for lt in range(NLT):
                    nc.tensor.matmul(
                        pn[:], wb[:, lt, :], qlat_t[:, lt, :],
                        start=(lt == 0), stop=(lt == NLT - 1),
                    )
                qnope = hwork.tile([P, CH], BF16, tag="qnope")
                nc.scalar.add(qnope[:], pn[:], bqn_t[:, h:h + 1])

                # k_nope^T for this head: local compute or gathered load
                if h < H_GATH:
                    knopeT = hwork.tile([P, S], BF16, tag="knopeT")
                    for kc in range(4):
                        pk = ps2.tile([P, 512], F32, tag="proj", bufs=2)
                        for rc in range(4):
                            nc.tensor.matmul(
                                pk[:], kupT[:, rc, h * P:(h + 1) * P],
                                kfull[:, rc, kc * 512:(kc + 1) * 512],
                                start=(rc == 0), stop=(rc == 3),
                            )
                        nc.scalar.add(
                            knopeT[:, kc * 512:(kc + 1) * 512], pk[:],
                            bkn_t[:, h:h + 1],
                        )
                else:
                    knopeT = kn_pending
                if h + 1 >= H_GATH and h + 1 < H:
                    kn_pending = load_knope(h + 1)

                qropeT = hwork.tile([P, CH], BF16, tag="qropeT")
                nc.sync.dma_start(qropeT[0:32, :], qro[m * 32:(m + 1) * 32, 0, :])
                nc.sync.dma_start(qropeT[32:64, :], qro[m * 32:(m + 1) * 32, 1, :])
                nc.sync.dma_start(qropeT[64:96, :], qro[m * 32:(m + 1) * 32, 0, :])
                nc.sync.dma_start(qropeT[96:128, :], qro[m * 32:(m + 1) * 32, 1, :])

                # prefetch next group's gathered v_lat late (after its AG lands)
                if m == 2 and g + 1 >= G_GATH and g + 1 < G:
                    vl_pending = load_vlat(g + 1)

                probs = probs_p.tile([P, NKT, CH], BF16, tag="probs")
                folds = []
                quads = []
                octs = []
                pv = ps2.tile([P, 512], F32, tag="attn", bufs=1)
                pd = ps2.tile([P, 512], F32, tag="den", bufs=1)
                for p in range(NKT // 2):
                    kt, kt1 = 2 * p, 2 * p + 1
                    sc = ps2.tile([P, 2, 512], F32, tag="scores", bufs=2)
                    nc.tensor.matmul(
                        sc[:, 0, :], knopeT[:, kt * P:(kt + 1) * P], qnope[:],
                        start=True, stop=False,
                    )
                    nc.tensor.matmul(
                        sc[:, 1, :], knopeT[:, kt1 * P:(kt1 + 1) * P], qnope[:],
                        start=True, stop=False,
                    )
                    nc.tensor.matmul(
                        sc[:, 0, :], kfull[0:DR, 4, kt * P:(kt + 1) * P],
                        qropeT[0:DR, :], start=False, stop=True,
                    )
                    nc.tensor.matmul(
                        sc[:, 1, :], kfull[DR:P, 4, kt1 * P:(kt1 + 1) * P],
                        qropeT[DR:P, :], start=False, stop=True,
                        tile_position=(DR, 0),
                    )
                    nc.scalar.activation(probs[:, kt:kt + 2, :], sc[:], AF.Exp)
                    ft = foldp.tile([P, CH], F32R, tag="fold")
                    nc.vector.tensor_tensor(
                        ft[:], probs[:, kt, :], probs[:, kt1, :], OP.add
                    )
                    folds.append(ft)
                    if p % 2 == 1:
                        fq = foldp.tile([P, CH], F32R, tag="foldq")
                        nc.vector.tensor_tensor(
                            fq[:], folds[p - 1][:], folds[p][:], OP.add
                        )
                        quads.append(fq)
                    if p % 4 == 3:
                        fo = foldp.tile([P, CH], F32R, tag="foldo")
                        nc.vector.tensor_tensor(
                            fo[:], quads[-2][:], quads[-1][:], OP.add
                        )
                        octs.append(fo)
                    if p >= 1:
                        nc.tensor.matmul(
                            pv[:], vlatq[:, kt - 2, m * P:(m + 1) * P],
                            probs[:, kt - 2, :], start=(p == 1), stop=False,
                        )
                        nc.tensor.matmul(
                            pv[:], vlatq[:, kt - 1, m * P:(m + 1) * P],
                            probs[:, kt - 1, :], start=False, stop=False,
                        )
                    if p == 5:
                        nc.tensor.matmul(
                            pd[:], ones_t[:], octs[0][:],
                            start=True, stop=False,
                        )
                nc.tensor.matmul(
                    pv[:], vlatq[:, NKT - 2, m * P:(m + 1) * P],
                    probs[:, NKT - 2, :], start=False, stop=False,
                )
                nc.tensor.matmul(
                    pv[:], vlatq[:, NKT - 1, m * P:(m + 1) * P],
                    probs[:, NKT - 1, :], start=False, stop=True,
                )
                nc.tensor.matmul(
                    pd[:], ones_t[:], octs[1][:], start=False, stop=True,
                )
                recip = hwork.tile([P, CH], F32, tag="recip")
                nc.vector.reciprocal_approx_fast(recip[:], pd[:])
                nc.vector.tensor_tensor(avT[:, h, :], pv[:], recip[:], OP.mult)

        # ================== phase 3: o_proj in quarter passes ==================
        with ExitStack() as p3:
            outp = p3.enter_context(tc.tile_pool(name="outp", bufs=4))
            ps3 = p3.enter_context(tc.tile_pool(name="ps3", bufs=1, space="PSUM"))

            pre = wo_pre
            for quarter in range(4):
                po = ps3.tile([P, NQT, 512], F32, tag="po", bufs=2)
                for kt in range(H):
                    if kt < len(pre):
                        wo = pre[kt]
                    else:
                        wo = wop.tile([P, 512], BF16, tag="wo")
                        nc.sync.dma_start(
                            wo[:],
                            wo_t[kt * P:(kt + 1) * P,
                                 quarter * 512:(quarter + 1) * 512],
                        )
                    for qc in range(NQT):
                        nc.tensor.matmul(
                            po[:, qc, :],
                            avT[:, kt, qc * P:(qc + 1) * P],
                            wo[:],
                            start=(kt == 0), stop=(kt == H - 1),
                        )
                pre = []
                if quarter < 3:
                    for i in range(2):
                        wo = wop.tile([P, 512], BF16, tag="wo")
                        nc.sync.dma_start(
                            wo[:],
                            wo_t[i * P:(i + 1) * P,
                                 (quarter + 1) * 512:(quarter + 2) * 512],
                        )
                        pre.append(wo)
                for qc in range(NQT):
                    ot = outp.tile([P, 512], F32, tag="ot")
                    if qc % 2 == 0:
                        nc.vector.tensor_copy(ot[:], po[:, qc, :])
                    else:
                        nc.scalar.copy(ot[:], po[:, qc, :])
                    nc.sync.dma_start(
                        out_c[
                            qc * P:(qc + 1) * P,
                            quarter * 512:(quarter + 1) * 512,
                        ],
                        ot[:],
                    )

    nc.compile()
    return nc


_NC_CACHE = None


def _get_nc():
    global _NC_CACHE
    if _NC_CACHE is None:
        _NC_CACHE = build_nc()
    return _NC_CACHE


def _prep_in_maps(inputs):
    hidden = np.asarray(inputs["hidden_states"], dtype=np.float32)
    w_qa = np.asarray(inputs["w_qa"], dtype=np.float32)
    ln_qa_g = np.asarray(inputs["ln_qa_g"], dtype=np.float32)
    ln_qa_b = np.asarray(inputs["ln_qa_b"], dtype=np.float32)
    w_qb = np.asarray(inputs["w_qb"], dtype=np.float32)
    w_qrope = np.asarray(inputs["w_qrope"], dtype=np.float32)
    w_kva = np.asarray(inputs["w_kva"], dtype=np.float32)
    ln_kva_g = np.asarray(inputs["ln_kva_g"], dtype=np.float32)
    ln_kva_b = np.asarray(inputs["ln_kva_b"], dtype=np.float32)
    w_kvb = np.asarray(inputs["w_kvb"], dtype=np.float32)
    w_o = np.asarray(inputs["w_o"], dtype=np.float32)
    pos = np.asarray(inputs["position_ids"]).astype(np.int64)

    bf = bfloat16
    hidden_b = hidden.astype(bf)
    hst_all = [
        hidden_b[b].T.reshape(NDT, P, NKT // 2, 2, P).transpose(2, 0, 3, 1, 4)
        for b in range(B)
    ]
    wqa_t = np.ascontiguousarray(w_qa.T.astype(bf))
    # LN gamma folded into q up-projections; beta becomes an output bias:
    # q_nope = (ln0*g + b) @ w_qb.T = ln0 @ (w_qb*g).T + w_qb @ b
    wqb_g = w_qb * ln_qa_g[None, :]
    bqn = (w_qb @ ln_qa_b).astype(np.float32)
    wqb_t = np.ascontiguousarray(wqb_g.T.astype(bf))
    wqr_s = SCALE * w_qrope
    bqr_full = (wqr_s @ ln_qa_b).astype(np.float32)
    wqr_g = (wqr_s * ln_qa_g[None, :]).T
    wqr_t = np.ascontiguousarray(
        wqr_g.reshape(LAT, H, 2, DR // 2).transpose(0, 2, 1, 3)
        .reshape(LAT, H * DR).astype(bf)
    )
    bqr_perm = np.ascontiguousarray(
        bqr_full.reshape(H, 2, DR // 2).transpose(1, 0, 2).reshape(H * DR)
    )
    wkva_t = np.ascontiguousarray(w_kva.T.astype(bf))
    kup = (SCALE * w_kvb[: H * DN]).reshape(H, DN, R)
    bkn = (kup @ ln_kva_b).reshape(H * DN).astype(np.float32)
    kup_g = kup * ln_kva_g[None, None, :]
    kup_t = np.ascontiguousarray(
        kup_g.transpose(2, 0, 1).reshape(R, H * DN).astype(bf)
    )
    vup = w_kvb[H * DN:].reshape(H, DV, R)
    bvv = (vup @ ln_kva_b).reshape(H * DV).astype(np.float32)
    vup_g = vup * ln_kva_g[None, None, :]
    vup_t = np.ascontiguousarray(
        vup_g.transpose(2, 0, 1).reshape(R, H * DV).astype(bf)
    )
    wo_t = np.ascontiguousarray(w_o.T.astype(bf))
    ones_in = np.ones((P, P), dtype=np.float32)

    inv_freq = 1.0 / (10000.0 ** (np.arange(0, DR, 2, dtype=np.float64) / DR))
    ang = pos[:, None].astype(np.float64) * inv_freq[None, :]
    cosf = np.ascontiguousarray(np.cos(ang).astype(np.float32))
    sinf = np.ascontiguousarray(np.sin(ang).astype(np.float32))

    in_maps = []
    for c in range(N_CORES):
        b, ch = divmod(c, NQT)
        qs = ch * CH
        cq = np.ascontiguousarray(np.tile(cosf[qs:qs + CH, :].T, (NQT, 1)))
        sq = np.ascontiguousarray(np.tile(sinf[qs:qs + CH, :].T, (NQT, 1)))
        myp = [2 * ch, 2 * ch + 1]
        in_maps.append({
            "hsqt": np.ascontiguousarray(hst_all[b][myp]),
            "wqa_t": wqa_t,
            "wqb_t": wqb_t,
            "wqr_t": wqr_t,
            "wkva_t": wkva_t,
            "kup_t": kup_t,
            "vup_t": vup_t,
            "wo_t": wo_t,
            "bqn_v": bqn,
            "bqr_v": bqr_perm,
            "bkn_v": bkn,
            "bvv_v": bvv,
            "ones_in": ones_in,
            "ck_tab": np.ascontiguousarray(cosf[qs:qs + CH]),
            "sk_tab": np.ascontiguousarray(sinf[qs:qs + CH]),
            "cq_tab": cq,
            "sq_tab": sq,
        })
    return in_maps


def kernel(**inputs) -> np.ndarray:
    nc = _get_nc()
    in_maps = _prep_in_maps(inputs)
    res = run_bass_kernel_spmd(nc, in_maps, core_ids=list(range(N_CORES)))
    out = np.empty((B, S, D), dtype=np.float32)
    for c in range(N_CORES):
        b, ch = divmod(c, NQT)
        out[b, ch * CH:(ch + 1) * CH, :] = res.results[c]["out_c"]
    return out
